# revision 31
# baseline (speedup 1.0000x reference)
"""MLA decode kernel for 8 TRN2 NeuronCores (v4).

Sharding: batch-parallel, zero collectives - core b owns batch b. Each core
runs the full projection chain + attention over its batch's KV cache.

Key techniques:
- Score-path cache (kvT/peT) and wbk in fp8-e3m4 (value-path cache kvn
  stays bf16); wbk pre-scaled x64 (e3m4 normals start at 0.25), divided
  back on the copy-out.
- GEMVs kc-outer with one PSUM bank per 512-wide output chunk so the
  stationary x-column is reused across back-to-back matmuls.
- Minimal DMA-issue count (each dma_start costs ~600ns of engine time):
  consts in 2 blobs, wq_a+wkv_a merged into one stream, wq_b one stream,
  per-head staging rows batched into 3 scatter DMAs.
- Cache host-tiled to match SBUF layout: 8-16KB contiguous per partition,
  blocks 0-1 prefetched up front, 2-3 issued inside the attention loop.
- wo split: tiles prefetched during attention (scalar queue) + JIT (sync).
"""
import numpy as np
import ml_dtypes

import concourse.bacc as bacc
import concourse.mybir as mybir
from concourse import bass_utils
from concourse.tile import TileContext
from concourse.masks import make_identity

BF = mybir.dt.bfloat16
F8 = mybir.dt.float8e3
F32 = mybir.dt.float32
npbf = ml_dtypes.bfloat16
npf8 = ml_dtypes.float8_e3m4

N_CORES = 8
B, S, DIM = 8, 1, 2048
H = 16
QLR, KVLR = 1536, 512
DN, DR, DV = 128, 64, 128
TP = 8191
T = TP + 1                 # 8192
SCALE = float((DN + DR) ** -0.5)
EPS = 1e-6
W8 = 64.0                  # fp8 wbk pre-scale
TBW = 2048                 # t-block width
NTB = T // TBW             # 4 blocks
N_WOA = 1                  # wo tiles prefetched during attention

# bf16 const blob offsets (elements)
O_QNW = 0
O_KVNW = O_QNW + QLR
O_WQAB = O_KVNW + KVLR
O_WQBB = O_WQAB + QLR
O_WKVAB = O_WQBB + 3072
O_WOB = O_WKVAB + (KVLR + DR)
N_CBF = O_WOB + DIM

_NC_CACHE = {}


def _build():
    if "nc" in _NC_CACHE:
        return _NC_CACHE["nc"]
    nc = bacc.Bacc("TRN2", target_bir_lowering=False, debug=False,
                   num_devices=N_CORES)
    I = {}

    def inp(name, shape, dt):
        I[name] = nc.dram_tensor(name, shape, dt, kind="ExternalInput")
        return I[name]

    inp("xcol", [128, 16], BF)
    inp("wqa_k", [2048, QLR + KVLR + DR], BF)  # [kc*128+p, qa|kv|pe]
    inp("wqb_k", [QLR, 3072], BF)              # [kc*128+p, nope(2048)|pe]
    inp("wbk", [128, H * KVLR], F8)            # x64; [d, h*512+c]
    inp("wbv", [512, H * DV], BF)              # [cc*128+c, h*128+d]
    inp("wo_k", [2048, DIM], BF)               # rows kc*128+p over (h d)
    inp("kvT8", [512, T], F8)                  # [blk*128+p, cc*2048+tt]
    inp("pe8", [256, TBW], F8)                 # [blk*64+p, tt]
    inp("kvn2", [512, T], BF)                  # [blk*128+p, i*512+c]
    inp("cbf", [1, N_CBF], BF)                 # bias/norm blob
    inp("cf32", [1, 1024], F32)                # cos | sin
    out_d = nc.dram_tensor("out", [1, DIM], F32, kind="ExternalOutput")

    with TileContext(nc) as tc:
        _program(nc, tc, I, out_d)
    nc.compile()
    _NC_CACHE["nc"] = nc
    return nc


def _program(nc, tc, I, out_d):
    AL = mybir.AluOpType
    AF = mybir.ActivationFunctionType

    with (
        tc.tile_pool(name="consts", bufs=1) as cp,
        tc.tile_pool(name="wqa", bufs=4) as wqa_p,
        tc.tile_pool(name="wqb", bufs=2) as wqb_p,
        tc.tile_pool(name="wres", bufs=1) as wres,
        tc.tile_pool(name="woa", bufs=N_WOA) as woa_p,
        tc.tile_pool(name="wob_p", bufs=1) as wob_p,
        tc.tile_pool(name="kvt", bufs=2) as kvt_p,
        tc.tile_pool(name="pe", bufs=2) as pe_p,
        tc.tile_pool(name="kvn", bufs=2) as kvn_p,
        tc.tile_pool(name="attn", bufs=2) as atp,
        tc.tile_pool(name="ppg", bufs=1, space="PSUM") as ppg,
        tc.tile_pool(name="ppo", bufs=1, space="PSUM") as ppo,
        tc.tile_pool(name="ppt", bufs=2, space="PSUM") as ppt,
    ):
        id_bf = cp.tile([128, 128], BF)
        id_f = cp.tile([128, 128], F32)
        make_identity(nc, id_bf[:])
        make_identity(nc, id_f[:])

        xT = cp.tile([128, 16], BF)
        nc.sync.dma_start(out=xT[:], in_=I["xcol"].ap())
        cbf = cp.tile([1, N_CBF], BF)
        nc.sync.dma_start(out=cbf[:], in_=I["cbf"].ap())
        cf32 = cp.tile([1, 1024], F32)
        nc.sync.dma_start(out=cf32[:], in_=I["cf32"].ap())
        qnw = cbf[:, O_QNW:O_QNW + QLR]
        kvnw = cbf[:, O_KVNW:O_KVNW + KVLR]
        wqab = cbf[:, O_WQAB:O_WQAB + QLR]
        wqbb = cbf[:, O_WQBB:O_WQBB + 3072]
        wkvab = cbf[:, O_WKVAB:O_WKVAB + KVLR + DR]
        wob = cbf[:, O_WOB:O_WOB + DIM]
        cosq = cf32[:, 0:512]
        sinq = cf32[:, 512:1024]

        kvt_tiles, pe_tiles, kvn_tiles = [], [], []

        def issue_cache_block(blk):
            kvtt = kvt_p.tile([128, 4, TBW], F8, tag=f"kvt{blk % 2}",
                              name=f"kvt_{blk}")
            nc.gpsimd.dma_start(
                out=kvtt[:],
                in_=I["kvT8"].ap()[blk * 128:(blk + 1) * 128, :].rearrange(
                    "p (c t) -> p c t", c=4))
            pet = pe_p.tile([64, TBW], F8, tag=f"pe{blk % 2}",
                            name=f"pe_{blk}")
            nc.gpsimd.dma_start(
                out=pet[:], in_=I["pe8"].ap()[blk * 64:(blk + 1) * 64, :])
            kvt_tiles.append(kvtt)
            pe_tiles.append(pet)
            for half in range(2):
                kvnt = kvn_p.tile([128, 8, KVLR], BF, tag=f"kvn{half}",
                                  name=f"kvn_{blk}_{half}")
                nc.gpsimd.dma_start(
                    out=kvnt[:],
                    in_=I["kvn2"].ap()[blk * 128:(blk + 1) * 128,
                                       half * 4096:(half + 1) * 4096]
                    .rearrange("p (i c) -> p i c", i=8))
                kvn_tiles.append(kvnt)

        issue_cache_block(0)
        issue_cache_block(1)
        wbk_sb = wres.tile([128, H, KVLR], F8)
        nc.gpsimd.dma_start(
            out=wbk_sb[:],
            in_=I["wbk"].ap().rearrange("p (h c) -> p h c", h=H))
        wbv_sb = wres.tile([128, 4, H * DV], BF)
        nc.gpsimd.dma_start(
            out=wbv_sb[:],
            in_=I["wbv"].ap().rearrange("(n p) m -> p n m", p=128))

        # ---- rms helper: out_f32[1,N] ----
        def rmsnorm(in_view, N, w_view, tag, out_view):
            sq = cp.tile([1, 1536], F32, tag="scratch", name=f"sq{tag}")
            ssq = cp.tile([1, 1], F32, tag=f"ssq{tag}")
            nc.scalar.activation(out=sq[:, :N], in_=in_view, func=AF.Square,
                                 accum_out=ssq[:])
            ms = cp.tile([1, 1], F32, tag=f"ms{tag}")
            nc.vector.tensor_scalar(out=ms[:], in0=ssq[:], scalar1=1.0 / N,
                                    scalar2=EPS, op0=AL.mult, op1=AL.add)
            sd = cp.tile([1, 1], F32, tag=f"sd{tag}")
            nc.scalar.activation(out=sd[:], in_=ms[:], func=AF.Sqrt)
            rstd = cp.tile([1, 1], F32, tag=f"rstd{tag}")
            nc.vector.reciprocal(out=rstd[:], in_=sd[:])
            tmp = cp.tile([1, 1536], F32, tag="scratch", name=f"tmp{tag}")
            nc.vector.tensor_tensor(out=tmp[:, :N], in0=in_view,
                                    in1=w_view, op=AL.mult)
            nc.vector.tensor_scalar(out=out_view, in0=tmp[:, :N],
                                    scalar1=rstd[:], scalar2=None,
                                    op0=AL.mult)

        def trans_row(in_view, n, ps_out):
            nc.tensor.transpose(ps_out, in_view, id_f[0:1, 0:1])

        # ============ phase 1: wq_a + wkv_a (merged stream) ============
        g = [ppg.tile([16, 512], F32, tag=f"g{i}", name=f"g1_{i}")
             for i in range(5)]
        for kc in range(16):
            wt = wqa_p.tile([128, QLR + KVLR + DR], BF, tag="wqa",
                            name=f"wqa_{kc}")
            eng = nc.sync if kc % 2 == 0 else nc.scalar
            eng.dma_start(
                out=wt[:], in_=I["wqa_k"].ap()[kc * 128:(kc + 1) * 128, :])
            st = (kc == 0)
            sp = (kc == 15)
            for mb in range(4):
                nc.tensor.matmul(
                    g[mb][0:1, :], xT[:, kc:kc + 1],
                    wt[:, mb * 512:(mb + 1) * 512],
                    start=st, stop=sp, skip_group_check=True)
            nc.tensor.matmul(g[4][0:1, :64], xT[:, kc:kc + 1],
                             wt[:, 2048:2112],
                             start=st, stop=sp, skip_group_check=True)

        qa = cp.tile([1, 2048], F32, tag="bigA", name="qa")
        for mb in range(3):
            nc.vector.tensor_tensor(
                out=qa[:, mb * 512:(mb + 1) * 512], in0=g[mb][0:1, :],
                in1=wqab[:, mb * 512:(mb + 1) * 512], op=AL.add)
        kvpe = cp.tile([1, KVLR + DR], F32)
        nc.vector.tensor_tensor(out=kvpe[:, :512], in0=g[3][0:1, :],
                                in1=wkvab[:, :512], op=AL.add)
        nc.vector.tensor_tensor(out=kvpe[:, 512:], in0=g[4][0:1, :64],
                                in1=wkvab[:, 512:], op=AL.add)

        qan = cp.tile([1, QLR], F32, tag="bigB", name="qan")
        rmsnorm(qa[:, :QLR], QLR, qnw, "q", qan[:])

        # qan -> qanT [128, 12] bf16
        pt_qa = ppt.tile([128, 64], F32, tag="tr", name="pt_qa")
        for kc in range(12):
            trans_row(qan[:, kc * 128:(kc + 1) * 128], 128,
                      pt_qa[:, kc:kc + 1])
        qanT = cp.tile([128, 12], BF)
        nc.scalar.copy(out=qanT[:], in_=pt_qa[:, :12])

        # ============ phase 2: wq_b (merged nope|pe stream) ============
        g = [ppg.tile([16, 512], F32, tag=f"g{i}", name=f"g2_{i}")
             for i in range(5)]
        g.append(ppo.tile([16, 512], F32, tag="po", name="g2_5"))
        for kc in range(12):
            wt = wqb_p.tile([128, 3072], BF, tag="wqb", name=f"wqb_{kc}")
            eng = nc.sync if kc % 2 == 0 else nc.scalar
            eng.dma_start(
                out=wt[:], in_=I["wqb_k"].ap()[kc * 128:(kc + 1) * 128, :])
            st = (kc == 0)
            sp = (kc == 11)
            for mb in range(6):
                nc.tensor.matmul(
                    g[mb][0:1, :], qanT[:, kc:kc + 1],
                    wt[:, mb * 512:(mb + 1) * 512],
                    start=st, stop=sp, skip_group_check=True)

        qn_sb = cp.tile([1, H * DN], F32, tag="bigA", name="qn_sb")
        for mb in range(4):
            nc.vector.tensor_tensor(
                out=qn_sb[:, mb * 512:(mb + 1) * 512], in0=g[mb][0:1, :],
                in1=wqbb[:, mb * 512:(mb + 1) * 512], op=AL.add)
        qp_sb = cp.tile([1, H * DR], F32)
        for mb in range(2):
            nc.vector.tensor_tensor(
                out=qp_sb[:, mb * 512:(mb + 1) * 512],
                in0=g[4 + mb][0:1, :],
                in1=wqbb[:, 2048 + mb * 512:2048 + (mb + 1) * 512],
                op=AL.add)

        # rope(q_pe) on [1, h*64] layout
        qpv = qp_sb[:].rearrange("b (h r) -> b h r", h=H)
        xr = qpv[:, :, 0:64:2]
        xi = qpv[:, :, 1:64:2]
        cosv = cosq.rearrange("b (h j) -> b h j", h=H)
        sinv = sinq.rearrange("b (h j) -> b h j", h=H)
        rp = cp.tile([1, H * DR], F32, tag="bigB", name="rp")
        rpv = rp[:].rearrange("b (h r) -> b h r", h=H)
        s1 = cp.tile([1, 512], F32, tag="rs1")
        s2 = cp.tile([1, 512], F32, tag="rs2")
        s1v = s1[:].rearrange("b (h j) -> b h j", h=H)
        s2v = s2[:].rearrange("b (h j) -> b h j", h=H)
        nc.vector.tensor_tensor(out=s1v, in0=xi, in1=sinv, op=AL.mult)
        nc.vector.tensor_tensor(out=s2v, in0=xr, in1=cosv, op=AL.mult)
        nc.vector.tensor_tensor(out=rpv[:, :, 0:64:2], in0=s2v, in1=s1v,
                                op=AL.subtract)
        nc.vector.tensor_tensor(out=s1v, in0=xr, in1=sinv, op=AL.mult)
        nc.vector.tensor_tensor(out=s2v, in0=xi, in1=cosv, op=AL.mult)
        nc.vector.tensor_tensor(out=rpv[:, :, 1:64:2], in0=s1v, in1=s2v,
                                op=AL.add)

        # q_nope -> qnT [128, 16] bf16 ; q_pe -> qpT [64, 16] bf16
        pt_qn = ppt.tile([128, 64], F32, tag="tr", name="pt_qn")
        for h in range(H):
            trans_row(qn_sb[:, h * 128:(h + 1) * 128], 128,
                      pt_qn[:, h:h + 1])
        qnT = cp.tile([128, H], BF)
        nc.scalar.copy(out=qnT[:], in_=pt_qn[:, :H])
        pt_qp = ppt.tile([128, 64], F32, tag="tr", name="pt_qp")
        for h in range(H):
            trans_row(rp[:, h * 64:(h + 1) * 64], 64, pt_qp[:64, h:h + 1])
        qpT = cp.tile([64, H], BF)
        nc.scalar.copy(out=qpT[:], in_=pt_qp[:64, :H])

        # ============ phase 3: absorption ============
        qabs_sb = cp.tile([16, 512], BF)
        ga = [ppg.tile([16, 512], F32, tag=f"g{i}", name=f"ga_{i}")
              for i in range(4)]
        stg = [cp.tile([1, 8 * 512], BF, tag="stg", name=f"stg_{i}")
               for i in range(2)]
        for h in range(H):
            pa = ga[h % 4]
            nc.tensor.matmul(pa[0:1, :], qnT[:, h:h + 1],
                             wbk_sb[:, h, :], start=True, stop=True,
                             skip_group_check=True)
            dst = stg[h // 8][:, (h % 8) * 512:(h % 8 + 1) * 512]
            if h % 2 == 0:
                nc.scalar.activation(out=dst, in_=pa[0:1, :],
                                     func=AF.Copy, scale=1.0 / W8)
            else:
                nc.vector.tensor_scalar(out=dst, in0=pa[0:1, :],
                                        scalar1=1.0 / W8, scalar2=None,
                                        op0=AL.mult)
        for i in range(2):
            nc.sync.dma_start(
                out=qabs_sb[i * 8:(i + 1) * 8, :], in_=stg[i][:])
        pt_ab = ppt.tile([128, 64], BF, tag="tr", name="pt_ab")
        for cc in range(4):
            nc.tensor.transpose(pt_ab[:, cc * 16:(cc + 1) * 16],
                                qabs_sb[:, cc * 128:(cc + 1) * 128],
                                id_bf[0:H, 0:H])
        qT = cp.tile([128, 4, H], BF)
        nc.scalar.copy(out=qT[:], in_=pt_ab[:].rearrange(
            "p (c h) -> p c h", c=4))

        # ============ kv-new token ============
        kvn_f = cp.tile([1, KVLR], F32)
        rmsnorm(kvpe[:, :KVLR], KVLR, kvnw, "kv", kvn_f[:])
        kpe = cp.tile([1, DR], F32)
        kxr = kvpe[:, KVLR + 0:KVLR + 64:2]
        kxi = kvpe[:, KVLR + 1:KVLR + 64:2]
        ks1 = cp.tile([1, 32], F32, tag="krs1")
        ks2 = cp.tile([1, 32], F32, tag="krs2")
        nc.vector.tensor_tensor(out=ks1[:], in0=kxi, in1=sinq[:, :32],
                                op=AL.mult)
        nc.vector.tensor_tensor(out=ks2[:], in0=kxr, in1=cosq[:, :32],
                                op=AL.mult)
        nc.vector.tensor_tensor(out=kpe[:, 0:64:2], in0=ks2[:], in1=ks1[:],
                                op=AL.subtract)
        nc.vector.tensor_tensor(out=ks1[:], in0=kxr, in1=sinq[:, :32],
                                op=AL.mult)
        nc.vector.tensor_tensor(out=ks2[:], in0=kxi, in1=cosq[:, :32],
                                op=AL.mult)
        nc.vector.tensor_tensor(out=kpe[:, 1:64:2], in0=ks1[:], in1=ks2[:],
                                op=AL.add)

        kvn_bf = cp.tile([1, KVLR], BF)
        nc.scalar.copy(out=kvn_bf[:], in_=kvn_f[:])
        pt_kv = ppt.tile([128, 64], F32, tag="tr", name="pt_kv")
        for cc in range(4):
            trans_row(kvn_f[:, cc * 128:(cc + 1) * 128], 128,
                      pt_kv[:, cc:cc + 1])
        trans_row(kpe[:], 64, pt_kv[:64, 4:5])
        kvnT8 = cp.tile([128, 4], F8)
        nc.scalar.copy(out=kvnT8[:], in_=pt_kv[:, :4])
        kpeT8 = cp.tile([64, 1], F8)
        nc.scalar.copy(out=kpeT8[:], in_=pt_kv[:64, 4:5])

        # ============ phase 4: attention ============
        den = cp.tile([H, 16], F32)
        po = ppo.tile([H, 512], F32, tag="po", name="po")
        wo_tiles = []
        n_mm2 = NTB * 4 * 4
        pend = []

        def drain(item, mm2_i):
            dblk, dexs = item
            exTs = []

            def tr_one(s):
                ptr = ppt.tile([128, 64], BF, tag="tr",
                               name=f"ptr{dblk}_{s}")
                for u in range(4):
                    nc.tensor.transpose(ptr[:, u * 16:(u + 1) * 16],
                                        dexs[s][:, u * 128:(u + 1) * 128],
                                        id_bf[0:H, 0:H])
                exT = atp.tile([128, 64], BF, tag="expT",
                               name=f"exT{dblk}_{s}")
                nc.vector.tensor_copy(out=exT[:], in_=ptr[:])
                exTs.append(exT)

            tr_one(0)
            for s in range(4):
                if s + 1 < 4:
                    tr_one(s + 1)
                kvnt = kvn_tiles[dblk * 2 + s // 2]
                for u in range(4):
                    nc.tensor.matmul(
                        po[:], exTs[s][:, u * 16:(u + 1) * 16],
                        kvnt[:, (s % 2) * 4 + u, :],
                        start=(mm2_i == 0), stop=(mm2_i == n_mm2 - 1),
                        skip_group_check=True)
                    mm2_i += 1
            return mm2_i

        mm2_i = 0
        issue_cache_block(2)
        issue_cache_block(3)
        for blk in range(NTB):
            for kc in range(len(wo_tiles), min(N_WOA, (blk + 1))):
                wt = woa_p.tile([128, DIM], BF, tag="woa", name=f"woa_{kc}")
                nc.scalar.dma_start(
                    out=wt[:],
                    in_=I["wo_k"].ap()[kc * 128:(kc + 1) * 128, :])
                wo_tiles.append(wt)
            kvtt = kvt_tiles[blk]
            pet = pe_tiles[blk]
            if blk == NTB - 1:
                for cc in range(4):
                    nc.vector.tensor_copy(out=kvtt[:, cc, TBW - 1:TBW],
                                          in_=kvnT8[:, cc:cc + 1])
                nc.vector.tensor_copy(out=pet[:, TBW - 1:TBW], in_=kpeT8[:])
                nc.sync.dma_start(out=kvn_tiles[7][127:128, 7, :],
                                  in_=kvn_bf[0:1, :])
            # scores: per s-tile 5 matmuls then its exp, so Act starts
            # early; transposes+values of the PREVIOUS block run after this
            # block's scores (cross-block software pipeline)
            exs = []
            for s in range(4):
                scs = ppg.tile([16, 512], F32, tag=f"g{s}",
                               name=f"sc{blk}_{s}")
                for cc in range(4):
                    nc.tensor.matmul(
                        scs[:], qT[:, cc, :],
                        kvtt[:, cc, s * 512:(s + 1) * 512],
                        start=(cc == 0), stop=False, skip_group_check=True)
                nc.tensor.matmul(scs[:], qpT[:],
                                 pet[:, s * 512:(s + 1) * 512],
                                 start=False, stop=True,
                                 skip_group_check=True)
                ex = atp.tile([H, 512], BF, tag="exp", name=f"ex{blk}_{s}")
                nc.scalar.activation(
                    out=ex[:], in_=scs[:], func=AF.Exp, scale=SCALE,
                    accum_out=den[:, blk * 4 + s:blk * 4 + s + 1])
                exs.append(ex)
            mm2_i = drain((blk, exs), mm2_i)

        # ============ phase 5: normalize + wbv ============
        den1 = cp.tile([H, 1], F32)
        nc.vector.tensor_reduce(out=den1[:], in_=den[:],
                                axis=mybir.AxisListType.X, op=AL.add)
        dinv = cp.tile([H, 1], F32)
        nc.vector.reciprocal(out=dinv[:], in_=den1[:])
        oln = cp.tile([H, 512], BF)
        nc.vector.tensor_scalar(out=oln[:], in0=po[:], scalar1=dinv[:],
                                scalar2=None, op0=AL.mult)

        pt_o = ppt.tile([128, 64], BF, tag="tr", name="pt_o")
        for cc in range(4):
            nc.tensor.transpose(pt_o[:, cc * 16:(cc + 1) * 16],
                                oln[:, cc * 128:(cc + 1) * 128],
                                id_bf[0:H, 0:H])
        olT = cp.tile([128, 64], BF)
        nc.scalar.copy(out=olT[:], in_=pt_o[:])

        o_sb = cp.tile([16, 128], BF)
        gv = [ppg.tile([16, 512], F32, tag=f"g{i}", name=f"gv_{i}")
              for i in range(4)]
        stv = cp.tile([1, 16 * 128], BF, tag="stv", name="stv")
        for h in range(H):
            pv = gv[h % 4]
            for cc in range(4):
                nc.tensor.matmul(
                    pv[0:1, :128], olT[:, cc * 16 + h:cc * 16 + h + 1],
                    wbv_sb[:, cc, h * 128:(h + 1) * 128],
                    start=(cc == 0), stop=(cc == 3), skip_group_check=True)
            dst = stv[:, h * 128:(h + 1) * 128]
            if h % 2 == 0:
                nc.scalar.copy(out=dst, in_=pv[0:1, :128])
            else:
                nc.vector.tensor_copy(out=dst, in_=pv[0:1, :128])
        nc.sync.dma_start(out=o_sb[:], in_=stv[:])
        pt_oT = ppt.tile([128, 64], BF, tag="tr", name="pt_oT")
        nc.tensor.transpose(pt_oT[:, :16], o_sb[:], id_bf[0:H, 0:H])
        oT = cp.tile([128, H], BF)
        nc.scalar.copy(out=oT[:], in_=pt_oT[:, :16])

        # ============ phase 6: wo ============
        gw = [ppg.tile([16, 512], F32, tag=f"g{i}", name=f"gw_{i}")
              for i in range(4)]
        for kc in range(16):
            if kc < N_WOA:
                wt = wo_tiles[kc]
            else:
                wt = wob_p.tile([128, DIM], BF, tag="wob",
                                name=f"wob_{kc}")
                eng = (nc.sync, nc.scalar, nc.gpsimd)[kc % 3]
                eng.dma_start(
                    out=wt[:],
                    in_=I["wo_k"].ap()[kc * 128:(kc + 1) * 128, :])
            st = (kc == 0)
            sp = (kc == 15)
            for mb in range(4):
                nc.tensor.matmul(
                    gw[mb][0:1, :], oT[:, kc:kc + 1],
                    wt[:, mb * 512:(mb + 1) * 512],
                    start=st, stop=sp, skip_group_check=True)
        out_sb = cp.tile([1, DIM], F32, tag="bigA", name="out_sb")
        for mb in range(4):
            nc.vector.tensor_tensor(
                out=out_sb[:, mb * 512:(mb + 1) * 512], in0=gw[mb][0:1, :],
                in1=wob[:, mb * 512:(mb + 1) * 512], op=AL.add)
        nc.sync.dma_start(out=out_d.ap(), in_=out_sb[:])


def _prep_inputs(inputs):
    f = {k: np.asarray(v) for k, v in inputs.items()}
    x = f["x"].astype(np.float32).reshape(B, DIM)
    kvp = f["kv_cache_prefix"].astype(np.float32)
    pep_ = f["pe_cache_prefix"].astype(np.float32)
    cos = f["freqs_cos"].astype(np.float32).reshape(-1)[:32]
    sin = f["freqs_sin"].astype(np.float32).reshape(-1)[:32]

    wq_a = f["wq_a_w"].astype(np.float32)
    wq_b = f["wq_b_w"].astype(np.float32).reshape(H, DN + DR, QLR)
    wkv_a = f["wkv_a_w"].astype(np.float32)
    wkv_b = f["wkv_b_w"].astype(np.float32).reshape(H, DN + DV, KVLR)
    wo = f["wo_w"].astype(np.float32)
    wq_b_b = f["wq_b_b"].astype(np.float32).reshape(H, DN + DR)

    cbf = np.zeros(N_CBF, np.float32)
    cbf[O_QNW:O_QNW + QLR] = f["q_norm_w"].astype(np.float32)
    cbf[O_KVNW:O_KVNW + KVLR] = f["kv_norm_w"].astype(np.float32)
    cbf[O_WQAB:O_WQAB + QLR] = f["wq_a_b"].astype(np.float32)
    cbf[O_WQBB:O_WQBB + 2048] = wq_b_b[:, :DN].reshape(-1)
    cbf[O_WQBB + 2048:O_WQBB + 3072] = wq_b_b[:, DN:].reshape(-1)
    cbf[O_WKVAB:O_WKVAB + KVLR + DR] = f["wkv_a_b"].astype(np.float32)
    cbf[O_WOB:O_WOB + DIM] = f["wo_b"].astype(np.float32)
    cf32 = np.zeros((1, 1024), np.float32)
    cf32[0, :512] = np.tile(cos, H)
    cf32[0, 512:] = np.tile(sin, H)

    shared = {
        "wqa_k": np.ascontiguousarray(
            np.concatenate([wq_a.T, wkv_a.T], axis=1)).astype(npbf),
        "wqb_k": np.ascontiguousarray(np.concatenate(
            [wq_b[:, :DN].reshape(H * DN, QLR).T,
             wq_b[:, DN:].reshape(H * DR, QLR).T], axis=1)).astype(npbf),
        "wbk": np.ascontiguousarray(
            np.transpose(wkv_b[:, :DN], (1, 0, 2)).reshape(128, H * KVLR)
            * W8).astype(npf8),
        "wbv": np.ascontiguousarray(
            np.transpose(wkv_b[:, DN:].reshape(H, DV, 4, 128), (2, 3, 0, 1))
            .reshape(512, H * DV)).astype(npbf),
        "wo_k": np.ascontiguousarray(wo.T).astype(npbf),
        "cbf": cbf.reshape(1, N_CBF).astype(npbf),
        "cf32": cf32,
    }
    in_maps = []
    for b in range(B):
        kvfull = np.zeros((T, KVLR), np.float32)
        kvfull[:TP] = kvp[b]
        pefull = np.zeros((T, DR), np.float32)
        pefull[:TP] = pep_[b]
        m = dict(shared)
        m["xcol"] = np.ascontiguousarray(
            x[b].reshape(16, 128).T).astype(npbf)
        m["kvT8"] = np.ascontiguousarray(
            kvfull.reshape(4, TBW, 4, 128).transpose(0, 3, 2, 1)
            .reshape(512, T)).astype(npf8)
        m["pe8"] = np.ascontiguousarray(
            pefull.reshape(4, TBW, 64).transpose(0, 2, 1)
            .reshape(256, TBW)).astype(npf8)
        m["kvn2"] = np.ascontiguousarray(
            kvfull.reshape(4, 16, 128, KVLR).transpose(0, 2, 1, 3)
            .reshape(512, T)).astype(npbf)
        in_maps.append(m)
    return in_maps


def run(inputs, trace=False, tmpdir=None):
    nc = _build()
    in_maps = _prep_inputs(inputs)
    res = bass_utils.run_bass_kernel_spmd(
        nc, in_maps, core_ids=list(range(N_CORES)), trace=trace,
        tmpdir=tmpdir,
    )
    out = np.zeros((B, S, DIM), dtype=np.float32)
    for b in range(B):
        out[b, 0, :] = res.results[b]["out"][0]
    return out, res


def kernel(**inputs) -> np.ndarray:
    out, _ = run(inputs, trace=False)
    return out


# revision 32
# speedup vs baseline: 1.1943x; 1.1943x over previous
"""MLA decode kernel for 8 TRN2 NeuronCores (v4).

Sharding: batch-parallel, zero collectives - core b owns batch b. Each core
runs the full projection chain + attention over its batch's KV cache.

Key techniques:
- Score-path cache (kvT/peT) and wbk in fp8-e3m4 (value-path cache kvn
  stays bf16); wbk pre-scaled x64 (e3m4 normals start at 0.25), divided
  back on the copy-out.
- GEMVs kc-outer with one PSUM bank per 512-wide output chunk so the
  stationary x-column is reused across back-to-back matmuls.
- Minimal DMA-issue count (each dma_start costs ~600ns of engine time):
  consts in 2 blobs, wq_a+wkv_a merged into one stream, wq_b one stream,
  per-head staging rows batched into 3 scatter DMAs.
- Cache host-tiled to match SBUF layout: 8-16KB contiguous per partition,
  blocks 0-1 prefetched up front, 2-3 issued inside the attention loop.
- wo split: tiles prefetched during attention (scalar queue) + JIT (sync).
"""
import numpy as np
import ml_dtypes

import concourse.bacc as bacc
import concourse.mybir as mybir
from concourse import bass_utils
from concourse.tile import TileContext
from concourse.masks import make_identity

BF = mybir.dt.bfloat16
F8 = mybir.dt.float8e3
F32 = mybir.dt.float32
npbf = ml_dtypes.bfloat16
npf8 = ml_dtypes.float8_e3m4

N_CORES = 8
B, S, DIM = 8, 1, 2048
H = 16
QLR, KVLR = 1536, 512
DN, DR, DV = 128, 64, 128
TP = 8191
T = TP + 1                 # 8192
SCALE = float((DN + DR) ** -0.5)
EPS = 1e-6
W8 = 64.0                  # fp8 wbk pre-scale
TBW = 2048                 # t-block width
NTB = T // TBW             # 4 blocks
N_WOA = 3                  # wo tiles prefetched during attention

# bf16 const blob offsets (elements)
O_QNW = 0
O_KVNW = O_QNW + QLR
O_WQAB = O_KVNW + KVLR
O_WQBB = O_WQAB + QLR
O_WKVAB = O_WQBB + 3072
O_WOB = O_WKVAB + (KVLR + DR)
N_CBF = O_WOB + DIM

_NC_CACHE = {}


def _build():
    if "nc" in _NC_CACHE:
        return _NC_CACHE["nc"]
    nc = bacc.Bacc("TRN2", target_bir_lowering=False, debug=False,
                   num_devices=N_CORES)
    I = {}

    def inp(name, shape, dt):
        I[name] = nc.dram_tensor(name, shape, dt, kind="ExternalInput")
        return I[name]

    inp("xcol", [128, 16], BF)
    inp("wqa_k", [2048, QLR + KVLR + DR], BF)  # [kc*128+p, qa|kv|pe]
    inp("wqb_k", [QLR, 3072], BF)              # [kc*128+p, nope(2048)|pe]
    inp("wbk", [128, H * KVLR], F8)            # x64; [d, h*512+c]
    inp("wbv", [512, H * DV], BF)              # [cc*128+c, h*128+d]
    inp("wo_k", [2048, DIM], BF)               # rows kc*128+p over (h d)
    inp("kvT8", [512, T], F8)                  # [blk*128+p, cc*2048+tt]
    inp("pe8", [256, TBW], F8)                 # [blk*64+p, tt]
    inp("kvn2", [512, T], BF)                  # [blk*128+p, i*512+c]
    inp("cbf", [1, N_CBF], BF)                 # bias/norm blob
    inp("cf32", [1, 1024], F32)                # cos | sin
    out_d = nc.dram_tensor("out", [1, DIM], F32, kind="ExternalOutput")

    with TileContext(nc) as tc:
        _program(nc, tc, I, out_d)
    nc.compile()
    _NC_CACHE["nc"] = nc
    return nc


def _program(nc, tc, I, out_d):
    AL = mybir.AluOpType
    AF = mybir.ActivationFunctionType

    with (
        tc.tile_pool(name="consts", bufs=1) as cp,
        tc.tile_pool(name="wqa", bufs=4) as wqa_p,
        tc.tile_pool(name="wqb", bufs=2) as wqb_p,
        tc.tile_pool(name="wres", bufs=1) as wres,
        tc.tile_pool(name="woa", bufs=N_WOA) as woa_p,
        tc.tile_pool(name="wob_p", bufs=2) as wob_p,
        tc.tile_pool(name="kvt", bufs=1) as kvt_p,
        tc.tile_pool(name="pe", bufs=1) as pe_p,
        tc.tile_pool(name="kvn", bufs=2) as kvn_p,
        tc.tile_pool(name="attn", bufs=2) as atp,
        tc.tile_pool(name="ppg", bufs=1, space="PSUM") as ppg,
        tc.tile_pool(name="ppo", bufs=1, space="PSUM") as ppo,
        tc.tile_pool(name="ppt", bufs=2, space="PSUM") as ppt,
    ):
        id_bf = cp.tile([128, 128], BF)
        id_f = cp.tile([128, 128], F32)
        make_identity(nc, id_bf[:])
        make_identity(nc, id_f[:])

        xT = cp.tile([128, 16], BF)
        nc.sync.dma_start(out=xT[:], in_=I["xcol"].ap())
        cbf = cp.tile([1, N_CBF], BF)
        nc.sync.dma_start(out=cbf[:], in_=I["cbf"].ap())
        cf32 = cp.tile([1, 1024], F32)
        nc.sync.dma_start(out=cf32[:], in_=I["cf32"].ap())
        qnw = cbf[:, O_QNW:O_QNW + QLR]
        kvnw = cbf[:, O_KVNW:O_KVNW + KVLR]
        wqab = cbf[:, O_WQAB:O_WQAB + QLR]
        wqbb = cbf[:, O_WQBB:O_WQBB + 3072]
        wkvab = cbf[:, O_WKVAB:O_WKVAB + KVLR + DR]
        wob = cbf[:, O_WOB:O_WOB + DIM]
        cosq = cf32[:, 0:512]
        sinq = cf32[:, 512:1024]

        kvt_tiles, pe_tiles, kvn_tiles = [], [], []

        def issue_cache_block(blk):
            kvtt = kvt_p.tile([128, 4, TBW], F8, tag=f"kvt{blk % 2}",
                              name=f"kvt_{blk}")
            nc.gpsimd.dma_start(
                out=kvtt[:],
                in_=I["kvT8"].ap()[blk * 128:(blk + 1) * 128, :].rearrange(
                    "p (c t) -> p c t", c=4))
            pet = pe_p.tile([64, TBW], F8, tag=f"pe{blk % 2}",
                            name=f"pe_{blk}")
            nc.gpsimd.dma_start(
                out=pet[:], in_=I["pe8"].ap()[blk * 64:(blk + 1) * 64, :])
            kvt_tiles.append(kvtt)
            pe_tiles.append(pet)
            for half in range(2):
                kvnt = kvn_p.tile([128, 8, KVLR], BF, tag=f"kvn{half}",
                                  name=f"kvn_{blk}_{half}")
                nc.gpsimd.dma_start(
                    out=kvnt[:],
                    in_=I["kvn2"].ap()[blk * 128:(blk + 1) * 128,
                                       half * 4096:(half + 1) * 4096]
                    .rearrange("p (i c) -> p i c", i=8))
                kvn_tiles.append(kvnt)

        issue_cache_block(0)
        issue_cache_block(1)
        wbk_sb = wres.tile([128, H, KVLR], F8)
        nc.gpsimd.dma_start(
            out=wbk_sb[:],
            in_=I["wbk"].ap().rearrange("p (h c) -> p h c", h=H))
        wbv_sb = wres.tile([128, 4, H * DV], BF)
        nc.gpsimd.dma_start(
            out=wbv_sb[:],
            in_=I["wbv"].ap().rearrange("(n p) m -> p n m", p=128))

        # ---- rms helper: out_f32[1,N] ----
        def rmsnorm(in_view, N, w_view, tag, out_view):
            sq = cp.tile([1, 1536], F32, tag="scratch", name=f"sq{tag}")
            ssq = cp.tile([1, 1], F32, tag=f"ssq{tag}")
            nc.scalar.activation(out=sq[:, :N], in_=in_view, func=AF.Square,
                                 accum_out=ssq[:])
            ms = cp.tile([1, 1], F32, tag=f"ms{tag}")
            nc.vector.tensor_scalar(out=ms[:], in0=ssq[:], scalar1=1.0 / N,
                                    scalar2=EPS, op0=AL.mult, op1=AL.add)
            sd = cp.tile([1, 1], F32, tag=f"sd{tag}")
            nc.scalar.activation(out=sd[:], in_=ms[:], func=AF.Sqrt)
            rstd = cp.tile([1, 1], F32, tag=f"rstd{tag}")
            nc.vector.reciprocal(out=rstd[:], in_=sd[:])
            tmp = cp.tile([1, 1536], F32, tag="scratch", name=f"tmp{tag}")
            nc.vector.tensor_tensor(out=tmp[:, :N], in0=in_view,
                                    in1=w_view, op=AL.mult)
            nc.vector.tensor_scalar(out=out_view, in0=tmp[:, :N],
                                    scalar1=rstd[:], scalar2=None,
                                    op0=AL.mult)

        def trans_row(in_view, n, ps_out):
            nc.tensor.transpose(ps_out, in_view, id_f[0:1, 0:1])

        # ============ phase 1: wq_a + wkv_a (merged stream) ============
        g = [ppg.tile([16, 512], F32, tag=f"g{i}", name=f"g1_{i}")
             for i in range(5)]
        for kc in range(16):
            wt = wqa_p.tile([128, QLR + KVLR + DR], BF, tag="wqa",
                            name=f"wqa_{kc}")
            eng = nc.sync if kc % 2 == 0 else nc.scalar
            eng.dma_start(
                out=wt[:], in_=I["wqa_k"].ap()[kc * 128:(kc + 1) * 128, :])
            st = (kc == 0)
            sp = (kc == 15)
            for mb in range(4):
                nc.tensor.matmul(
                    g[mb][0:1, :], xT[:, kc:kc + 1],
                    wt[:, mb * 512:(mb + 1) * 512],
                    start=st, stop=sp, skip_group_check=True)
            nc.tensor.matmul(g[4][0:1, :64], xT[:, kc:kc + 1],
                             wt[:, 2048:2112],
                             start=st, stop=sp, skip_group_check=True)

        qa = cp.tile([1, 2048], F32, tag="bigA", name="qa")
        for mb in range(3):
            nc.vector.tensor_tensor(
                out=qa[:, mb * 512:(mb + 1) * 512], in0=g[mb][0:1, :],
                in1=wqab[:, mb * 512:(mb + 1) * 512], op=AL.add)
        kvpe = cp.tile([1, KVLR + DR], F32)
        nc.vector.tensor_tensor(out=kvpe[:, :512], in0=g[3][0:1, :],
                                in1=wkvab[:, :512], op=AL.add)
        nc.vector.tensor_tensor(out=kvpe[:, 512:], in0=g[4][0:1, :64],
                                in1=wkvab[:, 512:], op=AL.add)

        qan = cp.tile([1, QLR], F32, tag="bigB", name="qan")
        rmsnorm(qa[:, :QLR], QLR, qnw, "q", qan[:])

        # qan -> qanT [128, 12] bf16
        pt_qa = ppt.tile([128, 64], F32, tag="tr", name="pt_qa")
        for kc in range(12):
            trans_row(qan[:, kc * 128:(kc + 1) * 128], 128,
                      pt_qa[:, kc:kc + 1])
        qanT = cp.tile([128, 12], BF)
        nc.scalar.copy(out=qanT[:], in_=pt_qa[:, :12])

        # ============ phase 2: wq_b (merged nope|pe stream) ============
        g = [ppg.tile([16, 512], F32, tag=f"g{i}", name=f"g2_{i}")
             for i in range(5)]
        g.append(ppo.tile([16, 512], F32, tag="po", name="g2_5"))
        for kc in range(12):
            wt = wqb_p.tile([128, 3072], BF, tag="wqb", name=f"wqb_{kc}")
            eng = nc.sync if kc % 2 == 0 else nc.scalar
            eng.dma_start(
                out=wt[:], in_=I["wqb_k"].ap()[kc * 128:(kc + 1) * 128, :])
            st = (kc == 0)
            sp = (kc == 11)
            for mb in range(6):
                nc.tensor.matmul(
                    g[mb][0:1, :], qanT[:, kc:kc + 1],
                    wt[:, mb * 512:(mb + 1) * 512],
                    start=st, stop=sp, skip_group_check=True)

        qn_sb = cp.tile([1, H * DN], F32, tag="bigA", name="qn_sb")
        for mb in range(4):
            nc.vector.tensor_tensor(
                out=qn_sb[:, mb * 512:(mb + 1) * 512], in0=g[mb][0:1, :],
                in1=wqbb[:, mb * 512:(mb + 1) * 512], op=AL.add)
        qp_sb = cp.tile([1, H * DR], F32)
        for mb in range(2):
            nc.vector.tensor_tensor(
                out=qp_sb[:, mb * 512:(mb + 1) * 512],
                in0=g[4 + mb][0:1, :],
                in1=wqbb[:, 2048 + mb * 512:2048 + (mb + 1) * 512],
                op=AL.add)

        # rope(q_pe) on [1, h*64] layout
        qpv = qp_sb[:].rearrange("b (h r) -> b h r", h=H)
        xr = qpv[:, :, 0:64:2]
        xi = qpv[:, :, 1:64:2]
        cosv = cosq.rearrange("b (h j) -> b h j", h=H)
        sinv = sinq.rearrange("b (h j) -> b h j", h=H)
        rp = cp.tile([1, H * DR], F32, tag="bigB", name="rp")
        rpv = rp[:].rearrange("b (h r) -> b h r", h=H)
        s1 = cp.tile([1, 512], F32, tag="rs1")
        s2 = cp.tile([1, 512], F32, tag="rs2")
        s1v = s1[:].rearrange("b (h j) -> b h j", h=H)
        s2v = s2[:].rearrange("b (h j) -> b h j", h=H)
        nc.vector.tensor_tensor(out=s1v, in0=xi, in1=sinv, op=AL.mult)
        nc.vector.tensor_tensor(out=s2v, in0=xr, in1=cosv, op=AL.mult)
        nc.vector.tensor_tensor(out=rpv[:, :, 0:64:2], in0=s2v, in1=s1v,
                                op=AL.subtract)
        nc.vector.tensor_tensor(out=s1v, in0=xr, in1=sinv, op=AL.mult)
        nc.vector.tensor_tensor(out=s2v, in0=xi, in1=cosv, op=AL.mult)
        nc.vector.tensor_tensor(out=rpv[:, :, 1:64:2], in0=s1v, in1=s2v,
                                op=AL.add)

        # q_nope -> qnT [128, 16] bf16 ; q_pe -> qpT [64, 16] bf16
        pt_qn = ppt.tile([128, 64], F32, tag="tr", name="pt_qn")
        for h in range(H):
            trans_row(qn_sb[:, h * 128:(h + 1) * 128], 128,
                      pt_qn[:, h:h + 1])
        qnT = cp.tile([128, H], BF)
        nc.scalar.copy(out=qnT[:], in_=pt_qn[:, :H])
        pt_qp = ppt.tile([128, 64], F32, tag="tr", name="pt_qp")
        for h in range(H):
            trans_row(rp[:, h * 64:(h + 1) * 64], 64, pt_qp[:64, h:h + 1])
        qpT = cp.tile([64, H], BF)
        nc.scalar.copy(out=qpT[:], in_=pt_qp[:64, :H])

        # ============ phase 3: absorption ============
        qabs_sb = cp.tile([16, 512], BF)
        ga = [ppg.tile([16, 512], F32, tag=f"g{i}", name=f"ga_{i}")
              for i in range(4)]
        stg = [cp.tile([1, 8 * 512], BF, tag="stg", name=f"stg_{i}")
               for i in range(2)]
        for h in range(H):
            pa = ga[h % 4]
            nc.tensor.matmul(pa[0:1, :], qnT[:, h:h + 1],
                             wbk_sb[:, h, :], start=True, stop=True,
                             skip_group_check=True)
            dst = stg[h // 8][:, (h % 8) * 512:(h % 8 + 1) * 512]
            if h % 2 == 0:
                nc.scalar.activation(out=dst, in_=pa[0:1, :],
                                     func=AF.Copy, scale=1.0 / W8)
            else:
                nc.vector.tensor_scalar(out=dst, in0=pa[0:1, :],
                                        scalar1=1.0 / W8, scalar2=None,
                                        op0=AL.mult)
        for i in range(2):
            nc.sync.dma_start(
                out=qabs_sb[i * 8:(i + 1) * 8, :], in_=stg[i][:])
        pt_ab = ppt.tile([128, 64], BF, tag="tr", name="pt_ab")
        for cc in range(4):
            nc.tensor.transpose(pt_ab[:, cc * 16:(cc + 1) * 16],
                                qabs_sb[:, cc * 128:(cc + 1) * 128],
                                id_bf[0:H, 0:H])
        qT = cp.tile([128, 4, H], BF)
        nc.scalar.copy(out=qT[:], in_=pt_ab[:].rearrange(
            "p (c h) -> p c h", c=4))

        # ============ kv-new token ============
        kvn_f = cp.tile([1, KVLR], F32)
        rmsnorm(kvpe[:, :KVLR], KVLR, kvnw, "kv", kvn_f[:])
        kpe = cp.tile([1, DR], F32)
        kxr = kvpe[:, KVLR + 0:KVLR + 64:2]
        kxi = kvpe[:, KVLR + 1:KVLR + 64:2]
        ks1 = cp.tile([1, 32], F32, tag="krs1")
        ks2 = cp.tile([1, 32], F32, tag="krs2")
        nc.vector.tensor_tensor(out=ks1[:], in0=kxi, in1=sinq[:, :32],
                                op=AL.mult)
        nc.vector.tensor_tensor(out=ks2[:], in0=kxr, in1=cosq[:, :32],
                                op=AL.mult)
        nc.vector.tensor_tensor(out=kpe[:, 0:64:2], in0=ks2[:], in1=ks1[:],
                                op=AL.subtract)
        nc.vector.tensor_tensor(out=ks1[:], in0=kxr, in1=sinq[:, :32],
                                op=AL.mult)
        nc.vector.tensor_tensor(out=ks2[:], in0=kxi, in1=cosq[:, :32],
                                op=AL.mult)
        nc.vector.tensor_tensor(out=kpe[:, 1:64:2], in0=ks1[:], in1=ks2[:],
                                op=AL.add)

        kvn_bf = cp.tile([1, KVLR], BF)
        nc.scalar.copy(out=kvn_bf[:], in_=kvn_f[:])
        pt_kv = ppt.tile([128, 64], F32, tag="tr", name="pt_kv")
        for cc in range(4):
            trans_row(kvn_f[:, cc * 128:(cc + 1) * 128], 128,
                      pt_kv[:, cc:cc + 1])
        trans_row(kpe[:], 64, pt_kv[:64, 4:5])
        kvnT8 = cp.tile([128, 4], F8)
        nc.scalar.copy(out=kvnT8[:], in_=pt_kv[:, :4])
        kpeT8 = cp.tile([64, 1], F8)
        nc.scalar.copy(out=kpeT8[:], in_=pt_kv[:64, 4:5])

        # ============ phase 4: attention ============
        den = cp.tile([H, 16], F32)
        po = ppo.tile([H, 512], F32, tag="po", name="po")
        wo_tiles = []
        n_mm2 = NTB * 4 * 4
        pend = []

        def drain(item, mm2_i):
            dblk, dexs = item
            exTs = []

            def tr_one(s):
                ptr = ppt.tile([128, 64], BF, tag="tr",
                               name=f"ptr{dblk}_{s}")
                for u in range(4):
                    nc.tensor.transpose(ptr[:, u * 16:(u + 1) * 16],
                                        dexs[s][:, u * 128:(u + 1) * 128],
                                        id_bf[0:H, 0:H])
                exT = atp.tile([128, 64], BF, tag="expT",
                               name=f"exT{dblk}_{s}")
                nc.vector.tensor_copy(out=exT[:], in_=ptr[:])
                exTs.append(exT)

            tr_one(0)
            for s in range(4):
                if s + 1 < 4:
                    tr_one(s + 1)
                kvnt = kvn_tiles[dblk * 2 + s // 2]
                for u in range(4):
                    nc.tensor.matmul(
                        po[:], exTs[s][:, u * 16:(u + 1) * 16],
                        kvnt[:, (s % 2) * 4 + u, :],
                        start=(mm2_i == 0), stop=(mm2_i == n_mm2 - 1),
                        skip_group_check=True)
                    mm2_i += 1
            return mm2_i

        mm2_i = 0
        for blk in range(NTB):
            if blk < 2:
                issue_cache_block(blk + 2)
            for kc in range(len(wo_tiles), min(N_WOA, (blk + 1))):
                wt = woa_p.tile([128, DIM], BF, tag="woa", name=f"woa_{kc}")
                nc.scalar.dma_start(
                    out=wt[:],
                    in_=I["wo_k"].ap()[kc * 128:(kc + 1) * 128, :])
                wo_tiles.append(wt)
            kvtt = kvt_tiles[blk]
            pet = pe_tiles[blk]
            if blk == NTB - 1:
                for cc in range(4):
                    nc.vector.tensor_copy(out=kvtt[:, cc, TBW - 1:TBW],
                                          in_=kvnT8[:, cc:cc + 1])
                nc.vector.tensor_copy(out=pet[:, TBW - 1:TBW], in_=kpeT8[:])
                nc.sync.dma_start(out=kvn_tiles[7][127:128, 7, :],
                                  in_=kvn_bf[0:1, :])
            # scores: per s-tile 5 matmuls then its exp, so Act starts
            # early; transposes+values of the PREVIOUS block run after this
            # block's scores (cross-block software pipeline)
            exs = []
            for s in range(4):
                scs = ppg.tile([16, 512], F32, tag=f"g{s}",
                               name=f"sc{blk}_{s}")
                for cc in range(4):
                    nc.tensor.matmul(
                        scs[:], qT[:, cc, :],
                        kvtt[:, cc, s * 512:(s + 1) * 512],
                        start=(cc == 0), stop=False, skip_group_check=True)
                nc.tensor.matmul(scs[:], qpT[:],
                                 pet[:, s * 512:(s + 1) * 512],
                                 start=False, stop=True,
                                 skip_group_check=True)
                ex = atp.tile([H, 512], BF, tag="exp", name=f"ex{blk}_{s}")
                nc.scalar.activation(
                    out=ex[:], in_=scs[:], func=AF.Exp, scale=SCALE,
                    accum_out=den[:, blk * 4 + s:blk * 4 + s + 1])
                exs.append(ex)
            mm2_i = drain((blk, exs), mm2_i)

        # ============ phase 5: normalize + wbv ============
        den1 = cp.tile([H, 1], F32)
        nc.vector.tensor_reduce(out=den1[:], in_=den[:],
                                axis=mybir.AxisListType.X, op=AL.add)
        dinv = cp.tile([H, 1], F32)
        nc.vector.reciprocal(out=dinv[:], in_=den1[:])
        oln = cp.tile([H, 512], BF)
        nc.vector.tensor_scalar(out=oln[:], in0=po[:], scalar1=dinv[:],
                                scalar2=None, op0=AL.mult)

        pt_o = ppt.tile([128, 64], BF, tag="tr", name="pt_o")
        for cc in range(4):
            nc.tensor.transpose(pt_o[:, cc * 16:(cc + 1) * 16],
                                oln[:, cc * 128:(cc + 1) * 128],
                                id_bf[0:H, 0:H])
        olT = cp.tile([128, 64], BF)
        nc.scalar.copy(out=olT[:], in_=pt_o[:])

        o_sb = cp.tile([16, 128], BF)
        gv = [ppg.tile([16, 512], F32, tag=f"g{i}", name=f"gv_{i}")
              for i in range(4)]
        stv = cp.tile([1, 16 * 128], BF, tag="stv", name="stv")
        for h in range(H):
            pv = gv[h % 4]
            for cc in range(4):
                nc.tensor.matmul(
                    pv[0:1, :128], olT[:, cc * 16 + h:cc * 16 + h + 1],
                    wbv_sb[:, cc, h * 128:(h + 1) * 128],
                    start=(cc == 0), stop=(cc == 3), skip_group_check=True)
            dst = stv[:, h * 128:(h + 1) * 128]
            if h % 2 == 0:
                nc.scalar.copy(out=dst, in_=pv[0:1, :128])
            else:
                nc.vector.tensor_copy(out=dst, in_=pv[0:1, :128])
        nc.sync.dma_start(out=o_sb[:], in_=stv[:])
        pt_oT = ppt.tile([128, 64], BF, tag="tr", name="pt_oT")
        nc.tensor.transpose(pt_oT[:, :16], o_sb[:], id_bf[0:H, 0:H])
        oT = cp.tile([128, H], BF)
        nc.scalar.copy(out=oT[:], in_=pt_oT[:, :16])

        # ============ phase 6: wo ============
        gw = [ppg.tile([16, 512], F32, tag=f"g{i}", name=f"gw_{i}")
              for i in range(4)]
        for kc in range(16):
            if kc < N_WOA:
                wt = wo_tiles[kc]
            else:
                wt = wob_p.tile([128, DIM], BF, tag="wob",
                                name=f"wob_{kc}")
                eng = (nc.sync, nc.scalar, nc.gpsimd)[kc % 3]
                eng.dma_start(
                    out=wt[:],
                    in_=I["wo_k"].ap()[kc * 128:(kc + 1) * 128, :])
            st = (kc == 0)
            sp = (kc == 15)
            for mb in range(4):
                nc.tensor.matmul(
                    gw[mb][0:1, :], oT[:, kc:kc + 1],
                    wt[:, mb * 512:(mb + 1) * 512],
                    start=st, stop=sp, skip_group_check=True)
        out_sb = cp.tile([1, DIM], F32, tag="bigA", name="out_sb")
        for mb in range(4):
            nc.vector.tensor_tensor(
                out=out_sb[:, mb * 512:(mb + 1) * 512], in0=gw[mb][0:1, :],
                in1=wob[:, mb * 512:(mb + 1) * 512], op=AL.add)
        nc.sync.dma_start(out=out_d.ap(), in_=out_sb[:])


def _prep_inputs(inputs):
    f = {k: np.asarray(v) for k, v in inputs.items()}
    x = f["x"].astype(np.float32).reshape(B, DIM)
    kvp = f["kv_cache_prefix"].astype(np.float32)
    pep_ = f["pe_cache_prefix"].astype(np.float32)
    cos = f["freqs_cos"].astype(np.float32).reshape(-1)[:32]
    sin = f["freqs_sin"].astype(np.float32).reshape(-1)[:32]

    wq_a = f["wq_a_w"].astype(np.float32)
    wq_b = f["wq_b_w"].astype(np.float32).reshape(H, DN + DR, QLR)
    wkv_a = f["wkv_a_w"].astype(np.float32)
    wkv_b = f["wkv_b_w"].astype(np.float32).reshape(H, DN + DV, KVLR)
    wo = f["wo_w"].astype(np.float32)
    wq_b_b = f["wq_b_b"].astype(np.float32).reshape(H, DN + DR)

    cbf = np.zeros(N_CBF, np.float32)
    cbf[O_QNW:O_QNW + QLR] = f["q_norm_w"].astype(np.float32)
    cbf[O_KVNW:O_KVNW + KVLR] = f["kv_norm_w"].astype(np.float32)
    cbf[O_WQAB:O_WQAB + QLR] = f["wq_a_b"].astype(np.float32)
    cbf[O_WQBB:O_WQBB + 2048] = wq_b_b[:, :DN].reshape(-1)
    cbf[O_WQBB + 2048:O_WQBB + 3072] = wq_b_b[:, DN:].reshape(-1)
    cbf[O_WKVAB:O_WKVAB + KVLR + DR] = f["wkv_a_b"].astype(np.float32)
    cbf[O_WOB:O_WOB + DIM] = f["wo_b"].astype(np.float32)
    cf32 = np.zeros((1, 1024), np.float32)
    cf32[0, :512] = np.tile(cos, H)
    cf32[0, 512:] = np.tile(sin, H)

    shared = {
        "wqa_k": np.ascontiguousarray(
            np.concatenate([wq_a.T, wkv_a.T], axis=1)).astype(npbf),
        "wqb_k": np.ascontiguousarray(np.concatenate(
            [wq_b[:, :DN].reshape(H * DN, QLR).T,
             wq_b[:, DN:].reshape(H * DR, QLR).T], axis=1)).astype(npbf),
        "wbk": np.ascontiguousarray(
            np.transpose(wkv_b[:, :DN], (1, 0, 2)).reshape(128, H * KVLR)
            * W8).astype(npf8),
        "wbv": np.ascontiguousarray(
            np.transpose(wkv_b[:, DN:].reshape(H, DV, 4, 128), (2, 3, 0, 1))
            .reshape(512, H * DV)).astype(npbf),
        "wo_k": np.ascontiguousarray(wo.T).astype(npbf),
        "cbf": cbf.reshape(1, N_CBF).astype(npbf),
        "cf32": cf32,
    }
    in_maps = []
    for b in range(B):
        kvfull = np.zeros((T, KVLR), np.float32)
        kvfull[:TP] = kvp[b]
        pefull = np.zeros((T, DR), np.float32)
        pefull[:TP] = pep_[b]
        m = dict(shared)
        m["xcol"] = np.ascontiguousarray(
            x[b].reshape(16, 128).T).astype(npbf)
        m["kvT8"] = np.ascontiguousarray(
            kvfull.reshape(4, TBW, 4, 128).transpose(0, 3, 2, 1)
            .reshape(512, T)).astype(npf8)
        m["pe8"] = np.ascontiguousarray(
            pefull.reshape(4, TBW, 64).transpose(0, 2, 1)
            .reshape(256, TBW)).astype(npf8)
        m["kvn2"] = np.ascontiguousarray(
            kvfull.reshape(4, 16, 128, KVLR).transpose(0, 2, 1, 3)
            .reshape(512, T)).astype(npbf)
        in_maps.append(m)
    return in_maps


def run(inputs, trace=False, tmpdir=None):
    nc = _build()
    in_maps = _prep_inputs(inputs)
    res = bass_utils.run_bass_kernel_spmd(
        nc, in_maps, core_ids=list(range(N_CORES)), trace=trace,
        tmpdir=tmpdir,
    )
    out = np.zeros((B, S, DIM), dtype=np.float32)
    for b in range(B):
        out[b, 0, :] = res.results[b]["out"][0]
    return out, res


def kernel(**inputs) -> np.ndarray:
    out, _ = run(inputs, trace=False)
    return out


# revision 33
# speedup vs baseline: 1.1998x; 1.0046x over previous
"""MLA decode kernel for 8 TRN2 NeuronCores (v4).

Sharding: batch-parallel, zero collectives - core b owns batch b. Each core
runs the full projection chain + attention over its batch's KV cache.

Key techniques:
- Score-path cache (kvT/peT) and wbk in fp8-e3m4 (value-path cache kvn
  stays bf16); wbk pre-scaled x64 (e3m4 normals start at 0.25), divided
  back on the copy-out.
- GEMVs kc-outer with one PSUM bank per 512-wide output chunk so the
  stationary x-column is reused across back-to-back matmuls.
- Minimal DMA-issue count (each dma_start costs ~600ns of engine time):
  consts in 2 blobs, wq_a+wkv_a merged into one stream, wq_b one stream,
  per-head staging rows batched into 3 scatter DMAs.
- Cache host-tiled to match SBUF layout: 8-16KB contiguous per partition,
  blocks 0-1 prefetched up front, 2-3 issued inside the attention loop.
- wo split: tiles prefetched during attention (scalar queue) + JIT (sync).
"""
import numpy as np
import ml_dtypes

import concourse.bacc as bacc
import concourse.mybir as mybir
from concourse import bass_utils
from concourse.tile import TileContext
from concourse.masks import make_identity

BF = mybir.dt.bfloat16
F8 = mybir.dt.float8e3
F32 = mybir.dt.float32
npbf = ml_dtypes.bfloat16
npf8 = ml_dtypes.float8_e3m4

N_CORES = 8
B, S, DIM = 8, 1, 2048
H = 16
QLR, KVLR = 1536, 512
DN, DR, DV = 128, 64, 128
TP = 8191
T = TP + 1                 # 8192
SCALE = float((DN + DR) ** -0.5)
EPS = 1e-6
W8 = 64.0                  # fp8 wbk pre-scale
TBW = 2048                 # t-block width
NTB = T // TBW             # 4 blocks
N_WOA = 3                  # wo tiles prefetched during attention

# bf16 const blob offsets (elements)
O_QNW = 0
O_KVNW = O_QNW + QLR
O_WQAB = O_KVNW + KVLR
O_WQBB = O_WQAB + QLR
O_WKVAB = O_WQBB + 3072
O_WOB = O_WKVAB + (KVLR + DR)
N_CBF = O_WOB + DIM

_NC_CACHE = {}


def _build():
    if "nc" in _NC_CACHE:
        return _NC_CACHE["nc"]
    nc = bacc.Bacc("TRN2", target_bir_lowering=False, debug=False,
                   num_devices=N_CORES)
    I = {}

    def inp(name, shape, dt):
        I[name] = nc.dram_tensor(name, shape, dt, kind="ExternalInput")
        return I[name]

    inp("xcol", [128, 16], BF)
    inp("wqa_k", [2048, QLR + KVLR + DR], BF)  # [kc*128+p, qa|kv|pe]
    inp("wqb_k", [QLR, 3072], BF)              # [kc*128+p, nope(2048)|pe]
    inp("wbk", [128, H * KVLR], F8)            # x64; [d, h*512+c]
    inp("wbv", [512, H * DV], BF)              # [cc*128+c, h*128+d]
    inp("wo_k", [2048, DIM], BF)               # rows kc*128+p over (h d)
    inp("kvT8", [512, T], F8)                  # [blk*128+p, cc*2048+tt]
    inp("pe8", [256, TBW], F8)                 # [blk*64+p, tt]
    inp("kvn2", [512, T], BF)                  # [blk*128+p, i*512+c]
    inp("cbf", [1, N_CBF], BF)                 # bias/norm blob
    inp("cf32", [1, 1024], F32)                # cos | sin
    out_d = nc.dram_tensor("out", [1, DIM], F32, kind="ExternalOutput")

    with TileContext(nc) as tc:
        _program(nc, tc, I, out_d)
    nc.compile()
    _NC_CACHE["nc"] = nc
    return nc


def _program(nc, tc, I, out_d):
    AL = mybir.AluOpType
    AF = mybir.ActivationFunctionType

    with (
        tc.tile_pool(name="consts", bufs=1) as cp,
        tc.tile_pool(name="wqa", bufs=4) as wqa_p,
        tc.tile_pool(name="wqb", bufs=2) as wqb_p,
        tc.tile_pool(name="wres", bufs=1) as wres,
        tc.tile_pool(name="woa", bufs=N_WOA) as woa_p,
        tc.tile_pool(name="wob_p", bufs=2) as wob_p,
        tc.tile_pool(name="kvt", bufs=1) as kvt_p,
        tc.tile_pool(name="pe", bufs=1) as pe_p,
        tc.tile_pool(name="kvn", bufs=2) as kvn_p,
        tc.tile_pool(name="attn", bufs=2) as atp,
        tc.tile_pool(name="ppg", bufs=1, space="PSUM") as ppg,
        tc.tile_pool(name="ppo", bufs=1, space="PSUM") as ppo,
        tc.tile_pool(name="ppt", bufs=2, space="PSUM") as ppt,
    ):
        id_bf = cp.tile([128, 128], BF)
        id_f = cp.tile([128, 128], F32)
        make_identity(nc, id_bf[:])
        make_identity(nc, id_f[:])

        xT = cp.tile([128, 16], BF)
        nc.sync.dma_start(out=xT[:], in_=I["xcol"].ap())
        cbf = cp.tile([1, N_CBF], BF)
        nc.sync.dma_start(out=cbf[:], in_=I["cbf"].ap())
        cf32 = cp.tile([1, 1024], F32)
        nc.sync.dma_start(out=cf32[:], in_=I["cf32"].ap())
        qnw = cbf[:, O_QNW:O_QNW + QLR]
        kvnw = cbf[:, O_KVNW:O_KVNW + KVLR]
        wqab = cbf[:, O_WQAB:O_WQAB + QLR]
        wqbb = cbf[:, O_WQBB:O_WQBB + 3072]
        wkvab = cbf[:, O_WKVAB:O_WKVAB + KVLR + DR]
        wob = cbf[:, O_WOB:O_WOB + DIM]
        cosq = cf32[:, 0:512]
        sinq = cf32[:, 512:1024]

        kvt_tiles, pe_tiles, kvn_tiles = [], [], []

        def issue_cache_block(blk):
            kvtt = kvt_p.tile([128, 4, TBW], F8, tag=f"kvt{blk % 2}",
                              name=f"kvt_{blk}")
            nc.gpsimd.dma_start(
                out=kvtt[:],
                in_=I["kvT8"].ap()[blk * 128:(blk + 1) * 128, :].rearrange(
                    "p (c t) -> p c t", c=4))
            pet = pe_p.tile([64, TBW], F8, tag=f"pe{blk % 2}",
                            name=f"pe_{blk}")
            nc.gpsimd.dma_start(
                out=pet[:], in_=I["pe8"].ap()[blk * 64:(blk + 1) * 64, :])
            kvt_tiles.append(kvtt)
            pe_tiles.append(pet)
            for half in range(2):
                kvnt = kvn_p.tile([128, 8, KVLR], BF, tag=f"kvn{half}",
                                  name=f"kvn_{blk}_{half}")
                nc.gpsimd.dma_start(
                    out=kvnt[:],
                    in_=I["kvn2"].ap()[blk * 128:(blk + 1) * 128,
                                       half * 4096:(half + 1) * 4096]
                    .rearrange("p (i c) -> p i c", i=8))
                kvn_tiles.append(kvnt)

        issue_cache_block(0)
        issue_cache_block(1)
        wbk_sb = wres.tile([128, H, KVLR], F8)
        nc.gpsimd.dma_start(
            out=wbk_sb[:],
            in_=I["wbk"].ap().rearrange("p (h c) -> p h c", h=H))
        wbv_sb = wres.tile([128, 4, H * DV], BF)
        nc.gpsimd.dma_start(
            out=wbv_sb[:],
            in_=I["wbv"].ap().rearrange("(n p) m -> p n m", p=128))

        # ---- rms helper: out_f32[1,N] ----
        def rmsnorm(in_view, N, w_view, tag, out_view):
            sq = cp.tile([1, 1536], F32, tag="scratch", name=f"sq{tag}")
            ssq = cp.tile([1, 1], F32, tag=f"ssq{tag}")
            nc.scalar.activation(out=sq[:, :N], in_=in_view, func=AF.Square,
                                 accum_out=ssq[:])
            ms = cp.tile([1, 1], F32, tag=f"ms{tag}")
            nc.vector.tensor_scalar(out=ms[:], in0=ssq[:], scalar1=1.0 / N,
                                    scalar2=EPS, op0=AL.mult, op1=AL.add)
            sd = cp.tile([1, 1], F32, tag=f"sd{tag}")
            nc.scalar.activation(out=sd[:], in_=ms[:], func=AF.Sqrt)
            rstd = cp.tile([1, 1], F32, tag=f"rstd{tag}")
            nc.vector.reciprocal(out=rstd[:], in_=sd[:])
            tmp = cp.tile([1, 1536], F32, tag="scratch", name=f"tmp{tag}")
            nc.vector.tensor_tensor(out=tmp[:, :N], in0=in_view,
                                    in1=w_view, op=AL.mult)
            nc.vector.tensor_scalar(out=out_view, in0=tmp[:, :N],
                                    scalar1=rstd[:], scalar2=None,
                                    op0=AL.mult)

        def trans_row(in_view, n, ps_out):
            nc.tensor.transpose(ps_out, in_view, id_f[0:1, 0:1])

        # ============ phase 1: wq_a + wkv_a (merged stream) ============
        g = [ppg.tile([16, 512], F32, tag=f"g{i}", name=f"g1_{i}")
             for i in range(5)]
        for kc in range(16):
            wt = wqa_p.tile([128, QLR + KVLR + DR], BF, tag="wqa",
                            name=f"wqa_{kc}")
            eng = nc.sync if kc % 2 == 0 else nc.scalar
            eng.dma_start(
                out=wt[:], in_=I["wqa_k"].ap()[kc * 128:(kc + 1) * 128, :])
            st = (kc == 0)
            sp = (kc == 15)
            for mb in range(4):
                nc.tensor.matmul(
                    g[mb][0:1, :], xT[:, kc:kc + 1],
                    wt[:, mb * 512:(mb + 1) * 512],
                    start=st, stop=sp, skip_group_check=True)
            nc.tensor.matmul(g[4][0:1, :64], xT[:, kc:kc + 1],
                             wt[:, 2048:2112],
                             start=st, stop=sp, skip_group_check=True)

        qa = cp.tile([1, 2048], F32, tag="bigA", name="qa")
        for mb in range(3):
            nc.vector.tensor_tensor(
                out=qa[:, mb * 512:(mb + 1) * 512], in0=g[mb][0:1, :],
                in1=wqab[:, mb * 512:(mb + 1) * 512], op=AL.add)
        kvpe = cp.tile([1, KVLR + DR], F32)
        nc.vector.tensor_tensor(out=kvpe[:, :512], in0=g[3][0:1, :],
                                in1=wkvab[:, :512], op=AL.add)
        nc.vector.tensor_tensor(out=kvpe[:, 512:], in0=g[4][0:1, :64],
                                in1=wkvab[:, 512:], op=AL.add)

        qan = cp.tile([1, QLR], F32, tag="bigB", name="qan")
        rmsnorm(qa[:, :QLR], QLR, qnw, "q", qan[:])

        # qan -> qanT [128, 12] bf16
        pt_qa = ppt.tile([128, 64], F32, tag="tr", name="pt_qa")
        for kc in range(12):
            trans_row(qan[:, kc * 128:(kc + 1) * 128], 128,
                      pt_qa[:, kc:kc + 1])
        qanT = cp.tile([128, 12], BF)
        nc.scalar.copy(out=qanT[:], in_=pt_qa[:, :12])

        # ============ phase 2: wq_b (merged nope|pe stream) ============
        g = [ppg.tile([16, 512], F32, tag=f"g{i}", name=f"g2_{i}")
             for i in range(5)]
        g.append(ppo.tile([16, 512], F32, tag="po", name="g2_5"))
        for kc in range(12):
            wt = wqb_p.tile([128, 3072], BF, tag="wqb", name=f"wqb_{kc}")
            eng = nc.sync if kc % 2 == 0 else nc.scalar
            eng.dma_start(
                out=wt[:], in_=I["wqb_k"].ap()[kc * 128:(kc + 1) * 128, :])
            st = (kc == 0)
            sp = (kc == 11)
            for mb in range(6):
                nc.tensor.matmul(
                    g[mb][0:1, :], qanT[:, kc:kc + 1],
                    wt[:, mb * 512:(mb + 1) * 512],
                    start=st, stop=sp, skip_group_check=True)

        qn_sb = cp.tile([1, H * DN], F32, tag="bigA", name="qn_sb")
        for mb in range(4):
            nc.vector.tensor_tensor(
                out=qn_sb[:, mb * 512:(mb + 1) * 512], in0=g[mb][0:1, :],
                in1=wqbb[:, mb * 512:(mb + 1) * 512], op=AL.add)
        qp_sb = cp.tile([1, H * DR], F32)
        for mb in range(2):
            nc.vector.tensor_tensor(
                out=qp_sb[:, mb * 512:(mb + 1) * 512],
                in0=g[4 + mb][0:1, :],
                in1=wqbb[:, 2048 + mb * 512:2048 + (mb + 1) * 512],
                op=AL.add)

        # rope(q_pe) on [1, h*64] layout
        qpv = qp_sb[:].rearrange("b (h r) -> b h r", h=H)
        xr = qpv[:, :, 0:64:2]
        xi = qpv[:, :, 1:64:2]
        cosv = cosq.rearrange("b (h j) -> b h j", h=H)
        sinv = sinq.rearrange("b (h j) -> b h j", h=H)
        rp = cp.tile([1, H * DR], F32, tag="bigB", name="rp")
        rpv = rp[:].rearrange("b (h r) -> b h r", h=H)
        s1 = cp.tile([1, 512], F32, tag="rs1")
        s2 = cp.tile([1, 512], F32, tag="rs2")
        s1v = s1[:].rearrange("b (h j) -> b h j", h=H)
        s2v = s2[:].rearrange("b (h j) -> b h j", h=H)
        nc.vector.tensor_tensor(out=s1v, in0=xi, in1=sinv, op=AL.mult)
        nc.vector.tensor_tensor(out=s2v, in0=xr, in1=cosv, op=AL.mult)
        nc.vector.tensor_tensor(out=rpv[:, :, 0:64:2], in0=s2v, in1=s1v,
                                op=AL.subtract)
        nc.vector.tensor_tensor(out=s1v, in0=xr, in1=sinv, op=AL.mult)
        nc.vector.tensor_tensor(out=s2v, in0=xi, in1=cosv, op=AL.mult)
        nc.vector.tensor_tensor(out=rpv[:, :, 1:64:2], in0=s1v, in1=s2v,
                                op=AL.add)

        # q_nope -> qnT [128, 16] bf16 ; q_pe -> qpT [64, 16] bf16
        pt_qn = ppt.tile([128, 64], F32, tag="tr", name="pt_qn")
        for h in range(H):
            trans_row(qn_sb[:, h * 128:(h + 1) * 128], 128,
                      pt_qn[:, h:h + 1])
        qnT = cp.tile([128, H], BF)
        nc.scalar.copy(out=qnT[:], in_=pt_qn[:, :H])
        pt_qp = ppt.tile([128, 64], F32, tag="tr", name="pt_qp")
        for h in range(H):
            trans_row(rp[:, h * 64:(h + 1) * 64], 64, pt_qp[:64, h:h + 1])
        qpT = cp.tile([64, H], BF)
        nc.scalar.copy(out=qpT[:], in_=pt_qp[:64, :H])

        # ============ phase 3: absorption ============
        qabs_sb = cp.tile([16, 512], BF)
        ga = [ppg.tile([16, 512], F32, tag=f"g{i}", name=f"ga_{i}")
              for i in range(4)]
        stg = [cp.tile([1, 8 * 512], BF, tag="stg", name=f"stg_{i}")
               for i in range(2)]
        for h in range(H):
            pa = ga[h % 4]
            nc.tensor.matmul(pa[0:1, :], qnT[:, h:h + 1],
                             wbk_sb[:, h, :], start=True, stop=True,
                             skip_group_check=True)
            dst = stg[h // 8][:, (h % 8) * 512:(h % 8 + 1) * 512]
            if h % 2 == 0:
                nc.scalar.activation(out=dst, in_=pa[0:1, :],
                                     func=AF.Copy, scale=1.0 / W8)
            else:
                nc.vector.tensor_scalar(out=dst, in0=pa[0:1, :],
                                        scalar1=1.0 / W8, scalar2=None,
                                        op0=AL.mult)
        for i in range(2):
            nc.sync.dma_start(
                out=qabs_sb[i * 8:(i + 1) * 8, :], in_=stg[i][:])
        pt_ab = ppt.tile([128, 64], BF, tag="tr", name="pt_ab")
        for cc in range(4):
            nc.tensor.transpose(pt_ab[:, cc * 16:(cc + 1) * 16],
                                qabs_sb[:, cc * 128:(cc + 1) * 128],
                                id_bf[0:H, 0:H])
        qT = cp.tile([128, 4, H], BF)
        nc.scalar.copy(out=qT[:], in_=pt_ab[:].rearrange(
            "p (c h) -> p c h", c=4))

        # ============ kv-new token ============
        kvn_f = cp.tile([1, KVLR], F32)
        rmsnorm(kvpe[:, :KVLR], KVLR, kvnw, "kv", kvn_f[:])
        kpe = cp.tile([1, DR], F32)
        kxr = kvpe[:, KVLR + 0:KVLR + 64:2]
        kxi = kvpe[:, KVLR + 1:KVLR + 64:2]
        ks1 = cp.tile([1, 32], F32, tag="krs1")
        ks2 = cp.tile([1, 32], F32, tag="krs2")
        nc.vector.tensor_tensor(out=ks1[:], in0=kxi, in1=sinq[:, :32],
                                op=AL.mult)
        nc.vector.tensor_tensor(out=ks2[:], in0=kxr, in1=cosq[:, :32],
                                op=AL.mult)
        nc.vector.tensor_tensor(out=kpe[:, 0:64:2], in0=ks2[:], in1=ks1[:],
                                op=AL.subtract)
        nc.vector.tensor_tensor(out=ks1[:], in0=kxr, in1=sinq[:, :32],
                                op=AL.mult)
        nc.vector.tensor_tensor(out=ks2[:], in0=kxi, in1=cosq[:, :32],
                                op=AL.mult)
        nc.vector.tensor_tensor(out=kpe[:, 1:64:2], in0=ks1[:], in1=ks2[:],
                                op=AL.add)

        kvn_bf = cp.tile([1, KVLR], BF)
        nc.scalar.copy(out=kvn_bf[:], in_=kvn_f[:])
        pt_kv = ppt.tile([128, 64], F32, tag="tr", name="pt_kv")
        for cc in range(4):
            trans_row(kvn_f[:, cc * 128:(cc + 1) * 128], 128,
                      pt_kv[:, cc:cc + 1])
        trans_row(kpe[:], 64, pt_kv[:64, 4:5])
        kvnT8 = cp.tile([128, 4], F8)
        nc.scalar.copy(out=kvnT8[:], in_=pt_kv[:, :4])
        kpeT8 = cp.tile([64, 1], F8)
        nc.scalar.copy(out=kpeT8[:], in_=pt_kv[:64, 4:5])

        # ============ phase 4: attention ============
        den = cp.tile([H, 16], F32)
        po = ppo.tile([H, 512], F32, tag="po", name="po")
        wo_tiles = []
        n_mm2 = NTB * 4 * 4
        pend = []

        def drain(item, mm2_i):
            dblk, dexs = item
            exTs = []

            def tr_one(s):
                ptr = ppt.tile([128, 64], BF, tag="tr",
                               name=f"ptr{dblk}_{s}")
                for u in range(4):
                    nc.tensor.transpose(ptr[:, u * 16:(u + 1) * 16],
                                        dexs[s][:, u * 128:(u + 1) * 128],
                                        id_bf[0:H, 0:H])
                exT = atp.tile([128, 64], BF, tag="expT",
                               name=f"exT{dblk}_{s}")
                nc.vector.tensor_copy(out=exT[:], in_=ptr[:])
                exTs.append(exT)

            tr_one(0)
            for s in range(4):
                if s + 1 < 4:
                    tr_one(s + 1)
                kvnt = kvn_tiles[dblk * 2 + s // 2]
                for u in range(4):
                    nc.tensor.matmul(
                        po[:], exTs[s][:, u * 16:(u + 1) * 16],
                        kvnt[:, (s % 2) * 4 + u, :],
                        start=(mm2_i == 0), stop=(mm2_i == n_mm2 - 1),
                        skip_group_check=True)
                    mm2_i += 1
            return mm2_i

        mm2_i = 0
        for blk in range(NTB):
            if blk < 2:
                issue_cache_block(blk + 2)
            for kc in range(len(wo_tiles), min(N_WOA, (blk + 1))):
                wt = woa_p.tile([128, DIM], BF, tag="woa", name=f"woa_{kc}")
                nc.scalar.dma_start(
                    out=wt[:],
                    in_=I["wo_k"].ap()[kc * 128:(kc + 1) * 128, :])
                wo_tiles.append(wt)
            kvtt = kvt_tiles[blk]
            pet = pe_tiles[blk]
            if blk == NTB - 1:
                for cc in range(4):
                    nc.vector.tensor_copy(out=kvtt[:, cc, TBW - 1:TBW],
                                          in_=kvnT8[:, cc:cc + 1])
                nc.vector.tensor_copy(out=pet[:, TBW - 1:TBW], in_=kpeT8[:])
                nc.sync.dma_start(out=kvn_tiles[7][127:128, 7, :],
                                  in_=kvn_bf[0:1, :])
            sc = [ppg.tile([16, 512], F32, tag=f"g{s}", name=f"sc{blk}_{s}")
                  for s in range(4)]
            for cc in range(4):
                for s in range(4):
                    nc.tensor.matmul(
                        sc[s][:], qT[:, cc, :],
                        kvtt[:, cc, s * 512:(s + 1) * 512],
                        start=(cc == 0), stop=False, skip_group_check=True)
            for s in range(4):
                nc.tensor.matmul(sc[s][:], qpT[:],
                                 pet[:, s * 512:(s + 1) * 512],
                                 start=False, stop=True,
                                 skip_group_check=True)
            exs = []
            for s in range(4):
                ex = atp.tile([H, 512], BF, tag="exp", name=f"ex{blk}_{s}")
                nc.scalar.activation(
                    out=ex[:], in_=sc[s][:], func=AF.Exp, scale=SCALE,
                    accum_out=den[:, blk * 4 + s:blk * 4 + s + 1])
                exs.append(ex)
            exTs = []
            for s in range(4):
                ptr = ppt.tile([128, 64], BF, tag="tr",
                               name=f"ptr{blk}_{s}")
                for u in range(4):
                    nc.tensor.transpose(ptr[:, u * 16:(u + 1) * 16],
                                        exs[s][:, u * 128:(u + 1) * 128],
                                        id_bf[0:H, 0:H])
                exT = atp.tile([128, 64], BF, tag="expT",
                               name=f"exT{blk}_{s}")
                nc.scalar.copy(out=exT[:], in_=ptr[:])
                exTs.append(exT)
            for s in range(4):
                kvnt = kvn_tiles[blk * 2 + s // 2]
                for u in range(4):
                    nc.tensor.matmul(
                        po[:], exTs[s][:, u * 16:(u + 1) * 16],
                        kvnt[:, (s % 2) * 4 + u, :],
                        start=(mm2_i == 0), stop=(mm2_i == n_mm2 - 1),
                        skip_group_check=True)
                    mm2_i += 1

        # ============ phase 5: normalize + wbv ============
        den1 = cp.tile([H, 1], F32)
        nc.vector.tensor_reduce(out=den1[:], in_=den[:],
                                axis=mybir.AxisListType.X, op=AL.add)
        dinv = cp.tile([H, 1], F32)
        nc.vector.reciprocal(out=dinv[:], in_=den1[:])
        oln = cp.tile([H, 512], BF)
        nc.vector.tensor_scalar(out=oln[:], in0=po[:], scalar1=dinv[:],
                                scalar2=None, op0=AL.mult)

        pt_o = ppt.tile([128, 64], BF, tag="tr", name="pt_o")
        for cc in range(4):
            nc.tensor.transpose(pt_o[:, cc * 16:(cc + 1) * 16],
                                oln[:, cc * 128:(cc + 1) * 128],
                                id_bf[0:H, 0:H])
        olT = cp.tile([128, 64], BF)
        nc.scalar.copy(out=olT[:], in_=pt_o[:])

        o_sb = cp.tile([16, 128], BF)
        gv = [ppg.tile([16, 512], F32, tag=f"g{i}", name=f"gv_{i}")
              for i in range(4)]
        stv = cp.tile([1, 16 * 128], BF, tag="stv", name="stv")
        for h in range(H):
            pv = gv[h % 4]
            for cc in range(4):
                nc.tensor.matmul(
                    pv[0:1, :128], olT[:, cc * 16 + h:cc * 16 + h + 1],
                    wbv_sb[:, cc, h * 128:(h + 1) * 128],
                    start=(cc == 0), stop=(cc == 3), skip_group_check=True)
            dst = stv[:, h * 128:(h + 1) * 128]
            if h % 2 == 0:
                nc.scalar.copy(out=dst, in_=pv[0:1, :128])
            else:
                nc.vector.tensor_copy(out=dst, in_=pv[0:1, :128])
        nc.sync.dma_start(out=o_sb[:], in_=stv[:])
        pt_oT = ppt.tile([128, 64], BF, tag="tr", name="pt_oT")
        nc.tensor.transpose(pt_oT[:, :16], o_sb[:], id_bf[0:H, 0:H])
        oT = cp.tile([128, H], BF)
        nc.scalar.copy(out=oT[:], in_=pt_oT[:, :16])

        # ============ phase 6: wo ============
        gw = [ppg.tile([16, 512], F32, tag=f"g{i}", name=f"gw_{i}")
              for i in range(4)]
        for kc in range(16):
            if kc < N_WOA:
                wt = wo_tiles[kc]
            else:
                wt = wob_p.tile([128, DIM], BF, tag="wob",
                                name=f"wob_{kc}")
                nc.sync.dma_start(
                    out=wt[:],
                    in_=I["wo_k"].ap()[kc * 128:(kc + 1) * 128, :])
            st = (kc == 0)
            sp = (kc == 15)
            for mb in range(4):
                nc.tensor.matmul(
                    gw[mb][0:1, :], oT[:, kc:kc + 1],
                    wt[:, mb * 512:(mb + 1) * 512],
                    start=st, stop=sp, skip_group_check=True)
        out_sb = cp.tile([1, DIM], F32, tag="bigA", name="out_sb")
        for mb in range(4):
            nc.vector.tensor_tensor(
                out=out_sb[:, mb * 512:(mb + 1) * 512], in0=gw[mb][0:1, :],
                in1=wob[:, mb * 512:(mb + 1) * 512], op=AL.add)
        nc.sync.dma_start(out=out_d.ap(), in_=out_sb[:])


def _prep_inputs(inputs):
    f = {k: np.asarray(v) for k, v in inputs.items()}
    x = f["x"].astype(np.float32).reshape(B, DIM)
    kvp = f["kv_cache_prefix"].astype(np.float32)
    pep_ = f["pe_cache_prefix"].astype(np.float32)
    cos = f["freqs_cos"].astype(np.float32).reshape(-1)[:32]
    sin = f["freqs_sin"].astype(np.float32).reshape(-1)[:32]

    wq_a = f["wq_a_w"].astype(np.float32)
    wq_b = f["wq_b_w"].astype(np.float32).reshape(H, DN + DR, QLR)
    wkv_a = f["wkv_a_w"].astype(np.float32)
    wkv_b = f["wkv_b_w"].astype(np.float32).reshape(H, DN + DV, KVLR)
    wo = f["wo_w"].astype(np.float32)
    wq_b_b = f["wq_b_b"].astype(np.float32).reshape(H, DN + DR)

    cbf = np.zeros(N_CBF, np.float32)
    cbf[O_QNW:O_QNW + QLR] = f["q_norm_w"].astype(np.float32)
    cbf[O_KVNW:O_KVNW + KVLR] = f["kv_norm_w"].astype(np.float32)
    cbf[O_WQAB:O_WQAB + QLR] = f["wq_a_b"].astype(np.float32)
    cbf[O_WQBB:O_WQBB + 2048] = wq_b_b[:, :DN].reshape(-1)
    cbf[O_WQBB + 2048:O_WQBB + 3072] = wq_b_b[:, DN:].reshape(-1)
    cbf[O_WKVAB:O_WKVAB + KVLR + DR] = f["wkv_a_b"].astype(np.float32)
    cbf[O_WOB:O_WOB + DIM] = f["wo_b"].astype(np.float32)
    cf32 = np.zeros((1, 1024), np.float32)
    cf32[0, :512] = np.tile(cos, H)
    cf32[0, 512:] = np.tile(sin, H)

    shared = {
        "wqa_k": np.ascontiguousarray(
            np.concatenate([wq_a.T, wkv_a.T], axis=1)).astype(npbf),
        "wqb_k": np.ascontiguousarray(np.concatenate(
            [wq_b[:, :DN].reshape(H * DN, QLR).T,
             wq_b[:, DN:].reshape(H * DR, QLR).T], axis=1)).astype(npbf),
        "wbk": np.ascontiguousarray(
            np.transpose(wkv_b[:, :DN], (1, 0, 2)).reshape(128, H * KVLR)
            * W8).astype(npf8),
        "wbv": np.ascontiguousarray(
            np.transpose(wkv_b[:, DN:].reshape(H, DV, 4, 128), (2, 3, 0, 1))
            .reshape(512, H * DV)).astype(npbf),
        "wo_k": np.ascontiguousarray(wo.T).astype(npbf),
        "cbf": cbf.reshape(1, N_CBF).astype(npbf),
        "cf32": cf32,
    }
    in_maps = []
    for b in range(B):
        kvfull = np.zeros((T, KVLR), np.float32)
        kvfull[:TP] = kvp[b]
        pefull = np.zeros((T, DR), np.float32)
        pefull[:TP] = pep_[b]
        m = dict(shared)
        m["xcol"] = np.ascontiguousarray(
            x[b].reshape(16, 128).T).astype(npbf)
        m["kvT8"] = np.ascontiguousarray(
            kvfull.reshape(4, TBW, 4, 128).transpose(0, 3, 2, 1)
            .reshape(512, T)).astype(npf8)
        m["pe8"] = np.ascontiguousarray(
            pefull.reshape(4, TBW, 64).transpose(0, 2, 1)
            .reshape(256, TBW)).astype(npf8)
        m["kvn2"] = np.ascontiguousarray(
            kvfull.reshape(4, 16, 128, KVLR).transpose(0, 2, 1, 3)
            .reshape(512, T)).astype(npbf)
        in_maps.append(m)
    return in_maps


def run(inputs, trace=False, tmpdir=None):
    nc = _build()
    in_maps = _prep_inputs(inputs)
    res = bass_utils.run_bass_kernel_spmd(
        nc, in_maps, core_ids=list(range(N_CORES)), trace=trace,
        tmpdir=tmpdir,
    )
    out = np.zeros((B, S, DIM), dtype=np.float32)
    for b in range(B):
        out[b, 0, :] = res.results[b]["out"][0]
    return out, res


def kernel(**inputs) -> np.ndarray:
    out, _ = run(inputs, trace=False)
    return out


# revision 34
# speedup vs baseline: 1.5556x; 1.2966x over previous
"""MLA decode kernel for 8 TRN2 NeuronCores.

Sharding: batch-parallel — core b handles batch element b (B=8, n_cores=8).
Each core runs the full projection chain (weights replicated, bf16,
host-pre-transposed into PE-friendly layouts) plus attention over its own
batch's KV cache. The KV cache is fed in BOTH layouts ([c,t] for the score
matmul and [t,c] for the value matmul) as bf16, so no on-chip transposes of
the big cache are needed and total cache DMA bytes equal one f32 copy.

All matmuls run in bf16 with f32 PSUM accumulation. Softmax skips the
max-subtraction (scores are O(3), exp is safe in f32) and normalizes after
the value matmul.
"""
import numpy as np
import ml_dtypes

import concourse.bacc as bacc
import concourse.mybir as mybir
from concourse import bass_utils
from concourse.tile import TileContext
from concourse.masks import make_identity

BF = mybir.dt.bfloat16
F32 = mybir.dt.float32
npbf = ml_dtypes.bfloat16

N_CORES = 8
B, S, DIM = 8, 1, 2048
H = 16
QLR, KVLR = 1536, 512
DN, DR, DV = 128, 64, 128
TP = 8191            # prefix length
T = TP + 1           # 8192 total positions
SCALE = float((DN + DR) ** -0.5)
EPS = 1e-6
TBW = 1024           # t-block width
NTB = T // TBW       # 4 blocks

_NC_CACHE = {}


def _chunked(ap_dram, p=128):
    # [K, M] dram AP -> [p, K//p, M] iteration view (partition-major)
    return ap_dram.rearrange("(n p) m -> p n m", p=p)


def _build():
    if "nc" in _NC_CACHE:
        return _NC_CACHE["nc"]
    nc = bacc.Bacc("TRN2", target_bir_lowering=False, debug=False,
                   num_devices=N_CORES)
    I = {}

    def inp(name, shape, dt=BF):
        I[name] = nc.dram_tensor(name, shape, dt, kind="ExternalInput")
        return I[name]

    inp("xT16", [128, 16])
    inp("kvT", [KVLR, TP])
    inp("kvn", [TP, KVLR])
    inp("peT", [DR, TP])
    inp("wqaT", [DIM, QLR])
    inp("wqbT", [QLR, H * (DN + DR)])
    inp("wkvaT", [DIM, KVLR + DR])
    inp("wbk", [H * DN, KVLR])        # (h,d) x c
    inp("wbv", [4 * 128, H * DV])     # (cc,c) x (h,d)
    inp("woT", [H * DV, DIM])
    inp("qnw", [1, QLR], F32)
    inp("kvnw", [1, KVLR], F32)
    inp("wqab", [1, QLR], F32)
    inp("wqbb", [1, H * (DN + DR)], F32)
    inp("wkvab", [1, KVLR + DR], F32)
    inp("wob", [1, DIM], F32)
    inp("cosq", [1, H * 32], F32)
    inp("sinq", [1, H * 32], F32)
    out_d = nc.dram_tensor("out", [1, DIM], F32, kind="ExternalOutput")

    with TileContext(nc) as tc:
        _program(nc, tc, I, out_d)
    nc.compile()
    _NC_CACHE["nc"] = nc
    return nc


def _program(nc, tc, I, out_d):
    AL = mybir.AluOpType
    AF = mybir.ActivationFunctionType

    with (
        tc.tile_pool(name="consts", bufs=1) as cp,
        tc.tile_pool(name="wstream", bufs=2) as wp,
        tc.tile_pool(name="wconst", bufs=1) as wc,
        tc.tile_pool(name="kvTp", bufs=3) as kvTp,
        tc.tile_pool(name="kvnp", bufs=2) as kvnp,
        tc.tile_pool(name="pep", bufs=2) as pep,
        tc.tile_pool(name="attn", bufs=3) as atp,
        tc.tile_pool(name="ps_scores", bufs=2, space="PSUM") as pps,
        tc.tile_pool(name="ps_acc", bufs=1, space="PSUM") as ppa,
        tc.tile_pool(name="ps_tr", bufs=2, space="PSUM") as ppt,
        tc.tile_pool(name="ps_stage", bufs=1, space="PSUM") as ppg,
    ):
        # identities for PE transpose
        id_bf = cp.tile([128, 128], BF)
        id_f = cp.tile([128, 128], F32)
        make_identity(nc, id_bf[:])
        make_identity(nc, id_f[:])

        def load_const(name, dt=F32):
            t = cp.tile(list(I[name].shape), dt, tag=name)
            nc.sync.dma_start(out=t[:], in_=I[name].ap())
            return t

        xT = load_const("xT16", BF)
        qnw = load_const("qnw")
        kvnw = load_const("kvnw")
        wqab = load_const("wqab")
        wqbb = load_const("wqbb")
        wkvab = load_const("wkvab")
        wob = load_const("wob")
        cosq = load_const("cosq")
        sinq = load_const("sinq")

        # ---- GEMV helper: y[1, M] f32 = xT_cols.T @ w  (+ bias) ----
        def gemv(xT_sb, nk, w_name, M, bias_sb, out_sb):
            wd = I[w_name].ap()
            for mb0 in range(0, M, 512):
                mw = min(512, M - mb0)
                wt = wp.tile([128, 16, 512], BF, tag="wstream")
                nc.sync.dma_start(
                    out=wt[:, :nk, :mw],
                    in_=_chunked(wd)[:, :, mb0:mb0 + mw],
                )
                ps = ppg.tile([1, 512], F32, tag="stage")
                for kc in range(nk):
                    nc.tensor.matmul(
                        ps[:, :mw], xT_sb[:, kc:kc + 1], wt[:, kc, :mw],
                        start=(kc == 0), stop=(kc == nk - 1),
                    )
                nc.vector.tensor_tensor(
                    out=out_sb[:, mb0:mb0 + mw], in0=ps[:, :mw],
                    in1=bias_sb[:, mb0:mb0 + mw], op=AL.add,
                )

        # ---- rms helper: out_f32[1,N] = in[1,N]*w*rsqrt(mean(in^2)+eps) ----
        def rmsnorm(in_view, N, w_sb, w_off, out_sb):
            sq = cp.tile([1, 1536], F32, tag="scratch")
            ssq = cp.tile([1, 1], F32, tag=f"ssq{N}_{w_off}")
            nc.scalar.activation(out=sq[:, :N], in_=in_view, func=AF.Square,
                                 accum_out=ssq[:])
            ms = cp.tile([1, 1], F32, tag=f"ms{N}_{w_off}")
            nc.vector.tensor_scalar(out=ms[:], in0=ssq[:], scalar1=1.0 / N,
                                    scalar2=EPS, op0=AL.mult, op1=AL.add)
            sd = cp.tile([1, 1], F32, tag=f"sd{N}_{w_off}")
            nc.scalar.activation(out=sd[:], in_=ms[:], func=AF.Sqrt)
            rstd = cp.tile([1, 1], F32, tag=f"rstd{N}_{w_off}")
            nc.vector.reciprocal(out=rstd[:], in_=sd[:])
            tmp = cp.tile([1, 1536], F32, tag="scratch")
            nc.vector.tensor_tensor(out=tmp[:, :N], in0=in_view,
                                    in1=w_sb[:, :N], op=AL.mult)
            nc.vector.tensor_scalar(out=out_sb, in0=tmp[:, :N],
                                    scalar1=rstd[:], scalar2=None, op0=AL.mult)

        # ---- transpose helper: [1, n] f32 row -> psum col [n, 1] ----
        def trans_row(in_view, n, ps_out):
            nc.tensor.transpose(ps_out, in_view, id_f[0:1, 0:1])

        # ================= Q branch =================
        qa = cp.tile([1, QLR], F32)
        gemv(xT, 16, "wqaT", QLR, wqab, qa)
        qan = cp.tile([1, QLR], F32)
        rmsnorm(qa[:], QLR, qnw, 0, qan[:])

        # transpose q_a_n -> [128, 12] bf16
        pt_qa = ppt.tile([128, 64], F32, tag="tr")
        for kc in range(12):
            trans_row(qan[:, kc * 128:(kc + 1) * 128], 128,
                      pt_qa[:, kc:kc + 1])
        qaT = cp.tile([128, 12], BF)
        nc.scalar.copy(out=qaT[:], in_=pt_qa[:, :12])

        q = cp.tile([1, H * (DN + DR)], F32)
        gemv(qaT, 12, "wqbT", H * (DN + DR), wqbb, q)

        # rope(q_pe): strided views over all 16 heads at once
        qv = q[:].rearrange("b (h r) -> b h r", h=H)
        xr = qv[:, :, 128:192:2]
        xi = qv[:, :, 129:192:2]
        cosv = cosq[:].rearrange("b (h j) -> b h j", h=H)
        sinv = sinq[:].rearrange("b (h j) -> b h j", h=H)
        rp = cp.tile([1, H * DR], F32)
        rpv = rp[:].rearrange("b (h r) -> b h r", h=H)
        s1 = cp.tile([1, 512], F32, tag="rs1")
        s2 = cp.tile([1, 512], F32, tag="rs2")
        s1v = s1[:].rearrange("b (h j) -> b h j", h=H)
        s2v = s2[:].rearrange("b (h j) -> b h j", h=H)
        nc.vector.tensor_tensor(out=s1v, in0=xi, in1=sinv, op=AL.mult)
        nc.vector.tensor_tensor(out=s2v, in0=xr, in1=cosv, op=AL.mult)
        nc.vector.tensor_tensor(out=rpv[:, :, 0:64:2], in0=s2v, in1=s1v,
                                op=AL.subtract)
        nc.vector.tensor_tensor(out=s1v, in0=xr, in1=sinv, op=AL.mult)
        nc.vector.tensor_tensor(out=s2v, in0=xi, in1=cosv, op=AL.mult)
        nc.vector.tensor_tensor(out=rpv[:, :, 1:64:2], in0=s1v, in1=s2v,
                                op=AL.add)

        # q_nope -> [128, 16] bf16 (transposed)
        pt_qn = ppt.tile([128, 64], F32, tag="tr")
        for h in range(H):
            trans_row(q[:, h * 192:h * 192 + 128], 128, pt_qn[:, h:h + 1])
        qnT = cp.tile([128, H], BF)
        nc.scalar.copy(out=qnT[:], in_=pt_qn[:, :H])

        # absorption: qT[cc][128c, 16h] bf16
        wbk_sb = wc.tile([128, H, KVLR], BF)
        nc.sync.dma_start(out=wbk_sb[:], in_=_chunked(I["wbk"].ap()))
        qT = []
        for cc in range(4):
            ps_ab = ppt.tile([128, 64], F32, tag="tr")
            for h in range(H):
                nc.tensor.matmul(
                    ps_ab[:, h:h + 1],
                    wbk_sb[:, h, cc * 128:(cc + 1) * 128],
                    qnT[:, h:h + 1], start=True, stop=True,
                )
            t = cp.tile([128, H], BF, tag=f"qT{cc}")
            nc.scalar.copy(out=t[:], in_=ps_ab[:, :H])
            qT.append(t)

        # q_pe transposed -> [64, 16] bf16
        pt_qp = ppt.tile([128, 64], F32, tag="tr")
        for h in range(H):
            trans_row(rp[:, h * 64:(h + 1) * 64], 64, pt_qp[:64, h:h + 1])
        qpT = cp.tile([64, H], BF)
        nc.scalar.copy(out=qpT[:], in_=pt_qp[:64, :H])

        # ================= KV branch =================
        kvpe = cp.tile([1, KVLR + DR], F32)
        gemv(xT, 16, "wkvaT", KVLR + DR, wkvab, kvpe)
        kvn_f = cp.tile([1, KVLR], F32)
        rmsnorm(kvpe[:, :KVLR], KVLR, kvnw, 1, kvn_f[:])

        # rope(k_pe) -> kpe [1, 64] f32
        kpe = cp.tile([1, DR], F32)
        kxr = kvpe[:, KVLR + 0:KVLR + 64:2]
        kxi = kvpe[:, KVLR + 1:KVLR + 64:2]
        ks1 = cp.tile([1, 32], F32, tag="krs1")
        ks2 = cp.tile([1, 32], F32, tag="krs2")
        nc.vector.tensor_tensor(out=ks1[:], in0=kxi, in1=sinq[:, :32], op=AL.mult)
        nc.vector.tensor_tensor(out=ks2[:], in0=kxr, in1=cosq[:, :32], op=AL.mult)
        nc.vector.tensor_tensor(out=kpe[:, 0:64:2], in0=ks2[:], in1=ks1[:],
                                op=AL.subtract)
        nc.vector.tensor_tensor(out=ks1[:], in0=kxr, in1=sinq[:, :32], op=AL.mult)
        nc.vector.tensor_tensor(out=ks2[:], in0=kxi, in1=cosq[:, :32], op=AL.mult)
        nc.vector.tensor_tensor(out=kpe[:, 1:64:2], in0=ks1[:], in1=ks2[:],
                                op=AL.add)

        # new-token tiles: kv_n bf16 row, kv_nT cols, kpeT col
        kvn_bf = cp.tile([1, KVLR], BF)
        nc.scalar.copy(out=kvn_bf[:], in_=kvn_f[:])
        pt_kv = ppt.tile([128, 64], F32, tag="tr")
        for cc in range(4):
            trans_row(kvn_f[:, cc * 128:(cc + 1) * 128], 128,
                      pt_kv[:, cc:cc + 1])
        trans_row(kpe[:], 64, pt_kv[:64, 4:5])
        kvnT = cp.tile([128, 4], BF)
        nc.scalar.copy(out=kvnT[:], in_=pt_kv[:, :4])
        kpeT = cp.tile([64, 1], BF)
        nc.scalar.copy(out=kpeT[:], in_=pt_kv[:64, 4:5])

        # ================= attention =================
        den = cp.tile([H, 16], F32)
        po = ppa.tile([H, 512], F32)
        kvT_d = I["kvT"].ap()
        kvn_d = I["kvn"].ap()
        peT_d = I["peT"].ap()
        n_mm2 = NTB * 8
        mm2_i = 0
        for tb in range(NTB):
            t0 = tb * TBW
            w = TBW if tb < NTB - 1 else TBW - 1  # prefix cols available
            kvTt = kvTp.tile([128, 4, TBW], BF, tag="kvT")
            nc.sync.dma_start(
                out=kvTt[:, :, :w],
                in_=kvT_d[:, t0:t0 + w].rearrange("(n p) t -> p n t", p=128),
            )
            pet = pep.tile([64, TBW], BF, tag="pe")
            nc.sync.dma_start(out=pet[:, :w], in_=peT_d[:, t0:t0 + w])
            kvnt = kvnp.tile([128, 8, 512], BF, tag="kvn")
            if tb < NTB - 1:
                nc.sync.dma_start(
                    out=kvnt[:],
                    in_=kvn_d[t0:t0 + TBW, :].rearrange("(n p) m -> p n m",
                                                        p=128),
                )
            else:
                nc.sync.dma_start(
                    out=kvnt[:, :7, :],
                    in_=kvn_d[t0:t0 + 896, :].rearrange("(n p) m -> p n m",
                                                        p=128),
                )
                nc.sync.dma_start(
                    out=kvnt[:127, 7, :],
                    in_=kvn_d[t0 + 896:t0 + 1023, :],
                )
                # inject the new token (t = 8191)
                for cc in range(4):
                    nc.vector.tensor_copy(out=kvTt[:, cc, TBW - 1:TBW],
                                          in_=kvnT[:, cc:cc + 1])
                nc.vector.tensor_copy(out=pet[:, TBW - 1:TBW], in_=kpeT[:])
                nc.sync.dma_start(out=kvnt[127:128, 7, :],
                                  in_=kvn_bf[0:1, :])

            for s in range(2):
                ps = pps.tile([H, 512], F32, tag="scores")
                for cc in range(4):
                    nc.tensor.matmul(
                        ps[:], qT[cc], kvTt[:, cc, s * 512:(s + 1) * 512],
                        start=(cc == 0), stop=False,
                    )
                nc.tensor.matmul(ps[:], qpT[:], pet[:, s * 512:(s + 1) * 512],
                                 start=False, stop=True)
                ex = atp.tile([H, 512], BF, tag="exp")
                nc.scalar.activation(out=ex[:], in_=ps[:], func=AF.Exp,
                                     scale=SCALE,
                                     accum_out=den[:, tb * 2 + s:tb * 2 + s + 1])
                ptr = ppt.tile([128, 64], BF, tag="trb")
                for u in range(4):
                    nc.tensor.transpose(ptr[:, u * 16:(u + 1) * 16],
                                        ex[:, u * 128:(u + 1) * 128],
                                        id_bf[0:H, 0:H])
                exT = atp.tile([128, 64], BF, tag="expT")
                nc.scalar.copy(out=exT[:], in_=ptr[:])
                for u in range(4):
                    nc.tensor.matmul(
                        po[:], exT[:, u * 16:(u + 1) * 16],
                        kvnt[:, s * 4 + u, :],
                        start=(mm2_i == 0), stop=(mm2_i == n_mm2 - 1),
                        skip_group_check=True,
                    )
                    mm2_i += 1

        # softmax denominator + normalize
        den1 = cp.tile([H, 1], F32)
        nc.vector.tensor_reduce(out=den1[:], in_=den[:],
                                axis=mybir.AxisListType.X, op=AL.add)
        dinv = cp.tile([H, 1], F32)
        nc.vector.reciprocal(out=dinv[:], in_=den1[:])
        oln = cp.tile([H, 512], BF)
        nc.vector.tensor_scalar(out=oln[:], in0=po[:], scalar1=dinv[:],
                                scalar2=None, op0=AL.mult)

        # transpose o_lat -> [128, 4*16] bf16 (col = cc*16+h)
        pt_o = ppt.tile([128, 64], BF, tag="trb")
        for cc in range(4):
            nc.tensor.transpose(pt_o[:, cc * 16:(cc + 1) * 16],
                                oln[:, cc * 128:(cc + 1) * 128],
                                id_bf[0:H, 0:H])
        olT = cp.tile([128, 64], BF)
        nc.scalar.copy(out=olT[:], in_=pt_o[:])

        # V projection -> oT [128d, 16h] bf16
        wbv_sb = wc.tile([128, 4, H * DV], BF)
        nc.sync.dma_start(out=wbv_sb[:], in_=_chunked(I["wbv"].ap()))
        ps_vo = ppt.tile([128, 64], F32, tag="tr")
        for h in range(H):
            for cc in range(4):
                nc.tensor.matmul(
                    ps_vo[:, h:h + 1],
                    wbv_sb[:, cc, h * 128:(h + 1) * 128],
                    olT[:, cc * 16 + h:cc * 16 + h + 1],
                    start=(cc == 0), stop=(cc == 3),
                )
        oT = cp.tile([128, H], BF)
        nc.scalar.copy(out=oT[:], in_=ps_vo[:, :H])

        # wo projection -> out [1, 2048] f32
        out_sb = cp.tile([1, DIM], F32)
        woT_d = I["woT"].ap()
        for mb in range(4):
            wt = wp.tile([128, 16, 512], BF, tag="wstream")
            nc.sync.dma_start(
                out=wt[:],
                in_=_chunked(woT_d)[:, :, mb * 512:(mb + 1) * 512],
            )
            ps = ppg.tile([1, 512], F32, tag="stage")
            for h in range(H):
                nc.tensor.matmul(ps[:], oT[:, h:h + 1], wt[:, h, :],
                                 start=(h == 0), stop=(h == H - 1))
            nc.vector.tensor_tensor(
                out=out_sb[:, mb * 512:(mb + 1) * 512], in0=ps[:],
                in1=wob[:, mb * 512:(mb + 1) * 512], op=AL.add,
            )
        nc.sync.dma_start(out=out_d.ap(), in_=out_sb[:])


def _prep_inputs(inputs):
    f = {k: np.asarray(v) for k, v in inputs.items()}
    x = f["x"].astype(np.float32).reshape(B, DIM)
    kvp = f["kv_cache_prefix"].astype(np.float32)
    pep_ = f["pe_cache_prefix"].astype(np.float32)
    cos = f["freqs_cos"].astype(np.float32).reshape(-1)[:32]
    sin = f["freqs_sin"].astype(np.float32).reshape(-1)[:32]

    wq_a = f["wq_a_w"].astype(np.float32)
    wq_b = f["wq_b_w"].astype(np.float32)
    wkv_a = f["wkv_a_w"].astype(np.float32)
    wkv_b = f["wkv_b_w"].astype(np.float32).reshape(H, DN + DV, KVLR)
    wo = f["wo_w"].astype(np.float32)

    shared = {
        "wqaT": np.ascontiguousarray(wq_a.T).astype(npbf),
        "wqbT": np.ascontiguousarray(wq_b.T).astype(npbf),
        "wkvaT": np.ascontiguousarray(wkv_a.T).astype(npbf),
        "wbk": np.ascontiguousarray(wkv_b[:, :DN].reshape(H * DN, KVLR)).astype(npbf),
        "wbv": np.ascontiguousarray(
            np.transpose(wkv_b[:, DN:].reshape(H, DV, 4, 128), (2, 3, 0, 1))
            .reshape(512, H * DV)).astype(npbf),
        "woT": np.ascontiguousarray(wo.T).astype(npbf),
        "qnw": f["q_norm_w"].astype(np.float32).reshape(1, QLR),
        "kvnw": f["kv_norm_w"].astype(np.float32).reshape(1, KVLR),
        "wqab": f["wq_a_b"].astype(np.float32).reshape(1, QLR),
        "wqbb": f["wq_b_b"].astype(np.float32).reshape(1, H * (DN + DR)),
        "wkvab": f["wkv_a_b"].astype(np.float32).reshape(1, KVLR + DR),
        "wob": f["wo_b"].astype(np.float32).reshape(1, DIM),
        "cosq": np.tile(cos, H).reshape(1, H * 32),
        "sinq": np.tile(sin, H).reshape(1, H * 32),
    }
    in_maps = []
    for b in range(B):
        m = dict(shared)
        m["xT16"] = np.ascontiguousarray(x[b].reshape(16, 128).T).astype(npbf)
        m["kvT"] = np.ascontiguousarray(kvp[b].T).astype(npbf)
        m["kvn"] = np.ascontiguousarray(kvp[b]).astype(npbf)
        m["peT"] = np.ascontiguousarray(pep_[b].T).astype(npbf)
        in_maps.append(m)
    return in_maps


def run(inputs, trace=False, tmpdir=None):
    nc = _build()
    in_maps = _prep_inputs(inputs)
    res = bass_utils.run_bass_kernel_spmd(
        nc, in_maps, core_ids=list(range(N_CORES)), trace=trace, tmpdir=tmpdir,
    )
    out = np.zeros((B, S, DIM), dtype=np.float32)
    for b in range(B):
        out[b, 0, :] = res.results[b]["out"][0]
    return out, res


def kernel(**inputs) -> np.ndarray:
    out, _ = run(inputs, trace=False)
    return out

